# revision 1
# baseline (speedup 1.0000x reference)
"""BiLSTM-CRF loss kernel for 8x Trainium2 NeuronCores (Bass/Tile).

Sharding: data-parallel over batch (16 sentences per core). Each core runs the
identical SPMD program: embedding gather -> 2 BiLSTM layers (fwd+bwd scans
interleaved per tick) -> emissions -> CRF forward algorithm (exp-space with
periodic rescaling) + path-score numerator. Host sums the per-core partials.

Math notes (per-core, b=16, batch rows on partitions):
 - tanh(x) = 2*sigmoid(2x) - 1 everywhere, so one Sigmoid activation covers all
   four gates.  With h~ = h/2 and c~ = c/2:
     s = sigmoid(z'), z' row-scaled so s_g = sigmoid(2 z_g)
     u  = (s_g - 0.5) * s_i          ( = i*g/2 )
     c~ = s_f * c~_prev + u
     h~ = (sigmoid(4 c~) - 0.5) * s_o
   The factors of 2 are folded into the weights host-side.
 - CRF forward pass runs in exp space:  a_t = (Eexp^T a_{t-1}) .* exp(em_t),
   Eexp = exp(trans), with a partition-sum rescale every RESCALE steps whose
   log is accumulated.  logZ = ln(sum_j a_T exp(end_j)) + sum(ln rescales).
"""

import sys

sys.path.insert(0, "/opt/trn_rl_repo")

import contextlib

import numpy as np
import ml_dtypes

import concourse.bass as bass
import concourse.tile as tile
from concourse import bacc, mybir
from concourse.masks import make_identity
from concourse.bass_utils import run_bass_kernel_spmd

F32 = mybir.dt.float32
F32R = mybir.dt.float32r
BF16 = mybir.dt.bfloat16
I16 = mybir.dt.int16
AF = mybir.ActivationFunctionType
OP = mybir.AluOpType

NCORES = 8
B, T, E, H, K, V = 128, 512, 128, 128, 20, 30000
G4 = 4 * H          # 512
BL = B // NCORES    # 16 sentences per core
RESCALE = 8


def _mm(nc, out, lhsT, rhs, start, stop, fast=True):
    nc.tensor.matmul(out, lhsT, rhs, start=start, stop=stop)


def build(nt=T):
    """Build the SPMD program for sequence length nt (nt=T for real use)."""
    nc = bacc.Bacc("TRN2", target_bir_lowering=False, debug=False,
                   num_devices=NCORES)
    NTB = nt * BL   # flattened (t,b) count per core

    # ---- DRAM I/O ----
    embedb = nc.dram_tensor("embedb", [V, E], BF16, kind="ExternalInput")
    toks16 = nc.dram_tensor("toks16", [BL, nt], I16, kind="ExternalInput")
    tagsf = nc.dram_tensor("tagsf", [1, NTB], F32, kind="ExternalInput")  # b-major
    wihT0 = nc.dram_tensor("wihT0", [2, E, G4], F32R, kind="ExternalInput")
    whhT0 = nc.dram_tensor("whhT0", [2, H, G4], F32R, kind="ExternalInput")
    b0v = nc.dram_tensor("b0v", [2, 1, G4], F32R, kind="ExternalInput")
    wih1T = nc.dram_tensor("wih1T", [2, 2, H, G4], F32R, kind="ExternalInput")
    whh1T = nc.dram_tensor("whh1T", [2, H, G4], F32R, kind="ExternalInput")
    b1v = nc.dram_tensor("b1v", [2, 1, G4], F32R, kind="ExternalInput")
    woutT = nc.dram_tensor("woutT", [2, H, K], F32R, kind="ExternalInput")
    boutv = nc.dram_tensor("boutv", [K, 1], F32, kind="ExternalInput")
    transm = nc.dram_tensor("transm", [K, K], F32, kind="ExternalInput")
    startv = nc.dram_tensor("startv", [K, 1], F32, kind="ExternalInput")
    endv = nc.dram_tensor("endv", [K, 1], F32, kind="ExternalInput")
    outm = nc.dram_tensor("outm", [2, BL], F32, kind="ExternalOutput")

    with tile.TileContext(nc) as tc, contextlib.ExitStack() as ctx:
        big = ctx.enter_context(tc.tile_pool(name="big", bufs=1))
        wp = ctx.enter_context(tc.tile_pool(name="wp", bufs=1))
        work = ctx.enter_context(tc.tile_pool(name="work", bufs=3))
        stp = ctx.enter_context(tc.tile_pool(name="stp", bufs=2))

        # ---------------- P0: constants, weights, gather ----------------
        idx = wp.tile([128, nt], I16, tag="idx")
        nc.gpsimd.memset(idx[:], 0)
        nc.sync.dma_start(out=idx[0:BL, :], in_=toks16[:, :])

        def load_w(name, dram_ap, shape, dt=F32):
            t = wp.tile(shape, dt, tag=name)
            nc.sync.dma_start(out=t[:], in_=dram_ap)
            return t

        wih0_sb = [load_w(f"wih0_{d}", wihT0[d], [E, G4], F32R) for d in range(2)]
        whh0_sb = [load_w(f"whh0_{d}", whhT0[d], [H, G4], F32R) for d in range(2)]
        b0_sb = [load_w(f"b0_{d}", b0v[d], [1, G4], F32R) for d in range(2)]
        wih1_sb = [[load_w(f"wih1_{d}{h}", wih1T[d, h], [H, G4], F32R)
                    for h in range(2)] for d in range(2)]
        whh1_sb = [load_w(f"whh1_{d}", whh1T[d], [H, G4], F32R) for d in range(2)]
        b1_sb = [load_w(f"b1_{d}", b1v[d], [1, G4], F32R) for d in range(2)]
        wout_sb = [load_w(f"wout_{d}", woutT[d], [H, K], F32R) for d in range(2)]
        bout_sb = load_w("bout", boutv[:, :], [K, 1])
        trans_sb = load_w("trans", transm[:, :], [K, K])
        start_sb = load_w("start", startv[:, :], [K, 1])
        end_sb = load_w("end", endv[:, :], [K, 1])

        id16 = wp.tile([BL, BL], F32, tag="id16")
        make_identity(nc, id16[:])
        ones16f = wp.tile([1, BL], F32, tag="ones16f")
        nc.vector.memset(ones16f[:], 1.0)
        ones16 = wp.tile([1, BL], F32R, tag="ones16")
        nc.vector.tensor_copy(ones16[:], ones16f[:])
        ones20 = wp.tile([K, 1], F32, tag="ones20")
        nc.vector.memset(ones20[:], 1.0)
        ones2020 = wp.tile([K, K], F32, tag="ones2020")
        nc.vector.memset(ones2020[:], 1.0)
        iota20 = wp.tile([K, 1], mybir.dt.int32, tag="iota20i")
        nc.gpsimd.iota(iota20[:], pattern=[[0, 1]], base=0,
                       channel_multiplier=1)
        iota20f = wp.tile([K, 1], F32, tag="iota20f")
        nc.vector.tensor_copy(iota20f[:], iota20[:])
        eexp = wp.tile([K, K], F32, tag="eexp")
        nc.scalar.activation(eexp[:], trans_sb[:], AF.Exp)
        expstart = wp.tile([K, 1], F32, tag="expstart")
        nc.scalar.activation(expstart[:], start_sb[:], AF.Exp)
        expend = wp.tile([K, 1], F32, tag="expend")
        nc.scalar.activation(expend[:], end_sb[:], AF.Exp)

        # Embedding gather (+transpose): xg[128_E, NTB] bf16, col = t*BL+b
        import os
        xg = big.tile([128, 1, NTB], BF16, tag="bigB")
        if os.environ.get("KK_NO_GATHER"):
            nc.gpsimd.memset(xg[:], 0)
        else:
            GCH = 256  # idxs per gather (SWDGE descriptor-ring limit)
            for g in range(max(1, NTB // GCH)):
                cw = min(GCH, NTB)
                nc.gpsimd.dma_gather(
                    xg[:, :, g * cw:(g + 1) * cw], embedb[:, :],
                    idx[:, g * (cw // 16):(g + 1) * (cw // 16)],
                    cw, cw, E, transpose=True)
        xT = big.tile([128, NTB], F32R, tag="bigA")
        nc.vector.tensor_copy(xT[:], xg[:, 0, :])

        # Histories (feature-on-partition, t-major slices of width BL)
        h0T = [big.tile([H, NTB], F32R, tag=f"h0T{d}", name=f"h0T{d}")
               for d in range(2)]

        # ---------------- P1 / P2: the two BiLSTM layers ----------------
        def scan_layer(layer, hist_out):
            """One BiLSTM layer: fwd+bwd scans as two independent chains."""
            with tc.tile_pool(name=f"zp{layer}", bufs=2, space="PSUM") as zp, \
                 tc.tile_pool(name=f"tp{layer}", bufs=2, space="PSUM") as tp:
                cprev = []
                for d in range(2):
                    cp0 = stp.tile([BL, H], F32, tag=f"c{layer}{d}",
                                   name=f"c{layer}{d}")
                    nc.vector.memset(cp0[:], 0.0)
                    cprev.append(cp0)
                for n in range(nt):
                    tt = [n, nt - 1 - n]     # [fwd t, bwd t]
                    for d in range(2):
                        t_ = tt[d]
                        sl = slice(t_ * BL, (t_ + 1) * BL)
                        z = zp.tile([BL, G4], F32, tag=f"z{d}", name=f"z{d}")
                        if layer == 0:
                            _mm(nc, z[:], xT[:, sl], wih0_sb[d][:],
                                start=True, stop=False)
                        else:
                            _mm(nc, z[:], h0T[0][:, sl], wih1_sb[d][0][:],
                                start=True, stop=False)
                            _mm(nc, z[:], h0T[1][:, sl], wih1_sb[d][1][:],
                                start=False, stop=False)
                        wb = (whh0_sb, b0_sb) if layer == 0 else \
                             (whh1_sb, b1_sb)
                        _mm(nc, z[:], ones16[:], wb[1][d][:],
                            start=False, stop=(n == 0))
                        if n > 0:
                            tprev = tt[d] + (-1 if d == 0 else 1)
                            psl = slice(tprev * BL, (tprev + 1) * BL)
                            _mm(nc, z[:], hist_out[d][:, psl], wb[0][d][:],
                                start=False, stop=True)
                        # gates (one chain per direction); i,f,g sigmoid is
                        # on the critical path, o-gate sigmoid is not.
                        s = work.tile([BL, G4], F32, tag=f"s{d}",
                                      name=f"s{d}")
                        nc.scalar.activation(s[:], z[:], AF.Sigmoid)
                        si = s[:, 0 * H:1 * H]
                        sf = s[:, 1 * H:2 * H]
                        sg = s[:, 2 * H:3 * H]
                        so = s[:, 3 * H:4 * H]
                        u = work.tile([BL, H], F32, tag=f"u{d}", name=f"u{d}")
                        nc.vector.scalar_tensor_tensor(
                            u[:], sg, -0.5, si, OP.add, OP.mult)
                        fc = work.tile([BL, H], F32, tag=f"fc{d}",
                                       name=f"fc{d}")
                        nc.vector.tensor_tensor(fc[:], sf, cprev[d][:],
                                                OP.mult)
                        cnew = stp.tile([BL, H], F32, tag=f"c{layer}{d}",
                                        name=f"cn{layer}{d}")
                        nc.vector.tensor_tensor(cnew[:], fc[:], u[:], OP.add)
                        sc = work.tile([BL, H], F32, tag=f"sc{d}",
                                       name=f"sc{d}")
                        nc.scalar.activation(sc[:], cnew[:], AF.Sigmoid,
                                             scale=4.0)
                        hb = work.tile([BL, H], F32, tag=f"hb{d}",
                                       name=f"hb{d}")
                        nc.vector.scalar_tensor_tensor(
                            hb[:], sc[:], -0.5, so, OP.add, OP.mult)
                        ht = tp.tile([H, BL], F32, tag=f"ht{d}",
                                     name=f"ht{d}")
                        nc.tensor.transpose(ht[:], hb[:], id16[:])
                        nc.vector.tensor_copy(hist_out[d][:, sl], ht[:])
                        cprev[d] = cnew

        scan_layer(0, h0T)
        h1T = [big.tile([H, NTB], F32R, tag="bigA", name="h1T0"),
               big.tile([H, NTB], F32R, tag="bigB", name="h1T1")]
        scan_layer(1, h1T)

        # ---------------- P3a: emissions ----------------
        emr = big.tile([K, NTB], F32, tag="h0T0")     # b-major: col=b*nt+t
        expem = big.tile([K, NTB], F32, tag="h0T1")   # t-major: col=t*BL+b
        with tc.tile_pool(name="ep", bufs=2, space="PSUM") as ep:
            ECH = 512 if NTB % 512 == 0 else NTB
            etch = ECH // BL                          # t per chunk
            for c in range(NTB // ECH):
                pe = ep.tile([K, ECH], F32)
                sl = slice(c * ECH, (c + 1) * ECH)
                _mm(nc, pe[:], wout_sb[0][:], h1T[0][:, sl], True, False)
                _mm(nc, pe[:], wout_sb[1][:], h1T[1][:, sl], False, True)
                # write em (+bout) b-major via strided AP
                pe3 = pe.rearrange("p (t b) -> p t b", b=BL)
                emr3 = emr.rearrange("p (b t) -> p b t", b=BL)[
                    :, :, c * etch:(c + 1) * etch].rearrange("p b t -> p t b")
                nc.scalar.activation(emr3, pe3, AF.Identity, bias=bout_sb[:])
        # exp(em) in t-major layout
        emr_tm = emr.rearrange("p (b t) -> p t b", b=BL)
        expem3 = expem.rearrange("p (t b) -> p t b", b=BL)
        nc.scalar.activation(expem3, emr_tm, AF.Exp)

        # ---------------- P3b: CRF forward (denominator) ----------------
        with tc.tile_pool(name="cp", bufs=1, space="PSUM") as cp, \
             tc.tile_pool(name="sp", bufs=1, space="PSUM") as sp, \
             tc.tile_pool(name="npp", bufs=2, space="PSUM") as npp:
            # two independent half-batch chains interleave to hide latency
            NH = 2
            HB = BL // NH
            aps, logaccs, pendings = [], [], []
            for hh in range(NH):
                hs = slice(hh * HB, (hh + 1) * HB)
                a0 = stp.tile([K, HB], F32, tag=f"alpha{hh}", name=f"a0_{hh}")
                nc.vector.tensor_tensor(
                    a0[:], expem[:, hs],
                    expstart[:, 0:1].to_broadcast([K, HB]), OP.mult)
                la0 = stp.tile([1, HB], F32, tag=f"logacc{hh}",
                               name=f"la0_{hh}")
                nc.vector.memset(la0[:], 0.0)
                aps.append(a0)
                logaccs.append(la0)
                pendings.append(None)
            for t_ in range(1, nt):
                for hh in range(NH):
                    hs = slice(t_ * BL + hh * HB, t_ * BL + (hh + 1) * HB)
                    pa = cp.tile([K, HB], F32, tag=f"pa{hh}", name=f"pa{hh}")
                    _mm(nc, pa[:], eexp[:], aps[hh][:], True, True,
                        fast=False)
                    an = stp.tile([K, HB], F32, tag=f"alpha{hh}",
                                  name=f"an{hh}")
                    nc.vector.tensor_tensor(an[:], pa[:], expem[:, hs],
                                            OP.mult)
                    aps[hh] = an
                    if pendings[hh] is not None and t_ >= pendings[hh][1]:
                        asc = stp.tile([K, HB], F32, tag=f"alpha{hh}",
                                       name=f"as{hh}")
                        nc.vector.tensor_tensor(
                            asc[:], aps[hh][:], pendings[hh][0][:], OP.mult)
                        aps[hh] = asc
                        pendings[hh] = None
                    if t_ % RESCALE == 0 and t_ + 2 < nt:
                        ps = sp.tile([K, HB], F32, tag=f"ps{hh}",
                                     name=f"ps{hh}")
                        _mm(nc, ps[:], ones2020[:], aps[hh][:], True, True,
                            fast=False)
                        sinv = work.tile([K, HB], F32, tag=f"sinv{hh}",
                                         name=f"sinv{hh}")
                        nc.vector.reciprocal(sinv[:], ps[:])
                        lt = work.tile([1, HB], F32, tag=f"lt{hh}",
                                       name=f"lt{hh}")
                        nc.scalar.activation(lt[:], ps[0:1, :], AF.Ln)
                        la = stp.tile([1, HB], F32, tag=f"logacc{hh}",
                                      name=f"lan{hh}")
                        nc.vector.tensor_tensor(la[:], logaccs[hh][:], lt[:],
                                                OP.add)
                        logaccs[hh] = la
                        pendings[hh] = (sinv, t_ + 2)
            logz = work.tile([1, BL], F32, tag="logz")
            for hh in range(NH):
                if pendings[hh] is not None:
                    asc = stp.tile([K, HB], F32, tag=f"alpha{hh}",
                                   name=f"af{hh}")
                    nc.vector.tensor_tensor(asc[:], aps[hh][:],
                                            pendings[hh][0][:], OP.mult)
                    aps[hh] = asc
                aend = work.tile([K, HB], F32, tag=f"aend{hh}",
                                 name=f"aend{hh}")
                nc.vector.tensor_tensor(
                    aend[:], aps[hh][:],
                    expend[:, 0:1].to_broadcast([K, HB]), OP.mult)
                psf = sp.tile([K, HB], F32, tag=f"ps{hh}", name=f"psf{hh}")
                _mm(nc, psf[:], ones2020[:], aend[:], True, True, fast=False)
                lnf = work.tile([1, HB], F32, tag=f"lnf{hh}",
                                name=f"lnf{hh}")
                nc.scalar.activation(lnf[:], psf[0:1, :], AF.Ln)
                nc.vector.tensor_tensor(
                    logz[:, hh * HB:(hh + 1) * HB], lnf[:], logaccs[hh][:],
                    OP.add)
            nc.sync.dma_start(out=outm[1:2, :], in_=logz[:])

            # ---------------- P3c: numerator (path score) ----------------
            tags_rep = big.tile([K, NTB], F32, tag="bigA", name="tags_rep")
            nc.sync.dma_start(out=tags_rep[:],
                              in_=tagsf[0:1, :].to_broadcast([K, NTB]))
            scol = stp.tile([K, BL], F32, tag="scol")
            spl = stp.tile([K, BL], F32, tag="spl")
            for b in range(BL):
                base = b * nt
                ohb = work.tile([K, nt], F32, tag="ohb")
                nc.vector.tensor_tensor(
                    ohb[:], iota20f[:, 0:1].to_broadcast([K, nt]),
                    tags_rep[:, base:base + nt], OP.is_equal)
                s1 = npp.tile([K, nt - 1], F32)
                _mm(nc, s1[:], trans_sb[:], ohb[:, 0:nt - 1], True, True)
                qa = work.tile([K, nt - 1], F32, tag="qa")
                nc.vector.tensor_tensor(
                    qa[:], s1[:], emr[:, base + 1:base + nt], OP.add)
                dump = work.tile([K, nt - 1], F32, tag="dump")
                nc.vector.scalar_tensor_tensor(
                    dump[:], qa[:], 0.0, ohb[:, 1:nt],
                    OP.add, OP.mult, accum_out=scol[:, b:b + 1])
                t0 = work.tile([K, 1], F32, tag="t0")
                nc.vector.scalar_tensor_tensor(
                    t0[:], emr[:, base:base + 1], start_sb[:, 0:1],
                    ohb[:, 0:1], OP.add, OP.mult)
                te = work.tile([K, 1], F32, tag="te")
                nc.vector.tensor_tensor(
                    te[:], ohb[:, nt - 1:nt], end_sb[:, 0:1], OP.mult)
                nc.vector.tensor_tensor(spl[:, b:b + 1], t0[:], te[:], OP.add)
            psc = sp.tile([K, BL], F32, tag="psc")
            _mm(nc, psc[:], ones2020[:], scol[:], True, False, fast=False)
            _mm(nc, psc[:], ones2020[:], spl[:], False, True, fast=False)
            score = work.tile([1, BL], F32, tag="score")
            nc.vector.tensor_copy(score[:], psc[0:1, :])
            nc.sync.dma_start(out=outm[0:1, :], in_=score[:])

    nc.compile()
    return nc


# ---------------------------------------------------------------------------
# Host side
# ---------------------------------------------------------------------------
_CACHE = {}


def _get_nc(nt):
    if nt not in _CACHE:
        _CACHE[nt] = build(nt)
    return _CACHE[nt]


def prep_inputs(sentences, tags, embed, Wih0, Whh0, b0, Wih1, Whh1, b1,
                Wout, bout, trans, start, end, nt=T):
    """Host-side marshalling: weight transposes + power-of-2 gate rescales."""
    f32 = np.float32
    sc = np.ones((G4, 1), f32)
    sc[2 * H:3 * H] = 2.0           # g rows: tanh-via-sigmoid needs 2x

    def stack2(w, s):
        return np.stack([np.ascontiguousarray((w[d] * s).T.astype(f32))
                         for d in range(2)])

    wihT0 = stack2(Wih0, sc)                    # [2,128,512] (transposed)
    whhT0 = stack2(Whh0, 2.0 * sc)
    b0v = np.stack([(b0[d][None, :] * sc[:, 0][None, :]).astype(f32)
                    for d in range(2)])
    wih1T_full = stack2(Wih1, 2.0 * sc)         # [2,256,512]
    wih1T = wih1T_full.reshape(2, 2, H, G4)
    whh1T = stack2(Whh1, 2.0 * sc)
    b1v = np.stack([(b1[d][None, :] * sc[:, 0][None, :]).astype(f32)
                    for d in range(2)])
    woutT = np.stack([np.ascontiguousarray((2.0 * Wout[:, :H]).T.astype(f32)),
                      np.ascontiguousarray((2.0 * Wout[:, H:]).T.astype(f32))])
    shared = dict(
        embedb=np.ascontiguousarray(embed.astype(ml_dtypes.bfloat16)),
        wihT0=wihT0, whhT0=whhT0, b0v=b0v, wih1T=wih1T, whh1T=whh1T, b1v=b1v,
        woutT=woutT, boutv=bout.reshape(K, 1).astype(f32),
        transm=trans.astype(f32), startv=start.reshape(K, 1).astype(f32),
        endv=end.reshape(K, 1).astype(f32),
    )
    in_maps = []
    for c in range(NCORES):
        bsl = slice(c * BL, (c + 1) * BL)
        m = dict(shared)
        m["toks16"] = np.ascontiguousarray(
            sentences[bsl, :nt].astype(np.int16))
        m["tagsf"] = np.ascontiguousarray(
            tags[bsl, :nt].astype(f32).reshape(1, BL * nt))
        in_maps.append(m)
    return in_maps


def run(inputs_np, nt=T, trace=False):
    nc = _get_nc(nt)
    in_maps = prep_inputs(
        inputs_np["sentences"], inputs_np["tags"], inputs_np["embed"],
        inputs_np["Wih0"], inputs_np["Whh0"], inputs_np["b0"],
        inputs_np["Wih1"], inputs_np["Whh1"], inputs_np["b1"],
        inputs_np["Wout"], inputs_np["bout"], inputs_np["trans"],
        inputs_np["start"], inputs_np["end"], nt=nt)
    res = run_bass_kernel_spmd(nc, in_maps, core_ids=list(range(NCORES)),
                               trace=trace)
    score = np.concatenate([res.results[c]["outm"][0] for c in range(NCORES)])
    logz = np.concatenate([res.results[c]["outm"][1] for c in range(NCORES)])
    loss = -np.mean(score - logz)
    return np.float32(loss), res


def kernel(**inputs):
    inputs_np = {k: np.asarray(v) for k, v in inputs.items()}
    loss, _ = run(inputs_np, nt=T)
    return np.asarray(loss, dtype=np.float32)



# revision 2
# speedup vs baseline: 1.8896x; 1.8896x over previous
"""BiLSTM-CRF loss kernel v2 for 8x Trainium2 NeuronCores (Bass/Tile).

Data-parallel over batch (16 sentences/core), feature-major layout:
hidden dim H=128 on partitions, batch in the free dim.

Per LSTM tick per direction the serial chain is only:
  4 tiny rec matmuls (bf16, 16 cols) -> sigmoid [128,64] (all 4 gates)
  -> u/fc/c~ (DVE+Pool) -> sigmoid(4c~) [128,16] -> h~ stt -> next matmul.
Input projections x@Wih + b are pre-accumulated into the same PSUM banks
8 ticks at a time by wide matmuls, so no per-tick add is needed.

Math identical to baseline: tanh via sigmoid (g-rows x2 in weights),
h~ = h/2 (x2 folded into Whh/Wih1/Wout), c~ = c/2, sc = sigmoid(4*c~).

CRF: exp-space with periodic rescaling, split into TWO concurrent chains
meeting in the middle:  Z = alpha_{M} . v_{M},  alpha forward from t=0,
v backward from t=T-1 (v_t = E @ (v_{t+1} * expem_{t+1})).

Numerator: one-hot tags OH [K,NTB]; G = Wout.h1+bout+trans^T@OH(shift)
accumulated in PSUM; score_b = sum_t <G(:,t,b), OH(:,t,b)> + end-term.
Emissions/numerator work drips into the layer-1 scan as chunks of h1
become ready.
"""

import sys

sys.path.insert(0, "/opt/trn_rl_repo")

import contextlib

import numpy as np
import ml_dtypes

import concourse.bass as bass
import concourse.tile as tile
from concourse import bacc, mybir
from concourse.bass_utils import run_bass_kernel_spmd

F32 = mybir.dt.float32
BF16 = mybir.dt.bfloat16
I16 = mybir.dt.int16
AF = mybir.ActivationFunctionType
OP = mybir.AluOpType

NCORES = 8
B, T, E, H, K, V = 128, 512, 128, 128, 20, 30000
BL = B // NCORES        # 16 sentences per core
GRP = 8                 # ticks per PSUM z-bank
CT = 32                 # ticks per emissions chunk (512 cols)
RESCALE = 8


def build(nt=T):
    nc = bacc.Bacc("TRN2", target_bir_lowering=False, debug=False,
                   num_devices=NCORES)
    NTB = nt * BL
    NG = nt // GRP
    NCH = NTB // (CT * BL)          # emissions chunks

    # ---- DRAM I/O ----
    embedb = nc.dram_tensor("embedb", [V, E], BF16, kind="ExternalInput")
    toks16 = nc.dram_tensor("toks16", [BL, nt], I16, kind="ExternalInput")
    tagsf = nc.dram_tensor("tagsf", [1, NTB], F32, kind="ExternalInput")  # t-major
    wih0m = nc.dram_tensor("wih0m", [2, 4, E, H], BF16, kind="ExternalInput")
    whh0m = nc.dram_tensor("whh0m", [2, 4, H, H], BF16, kind="ExternalInput")
    b0m = nc.dram_tensor("b0m", [2, 4, 1, H], BF16, kind="ExternalInput")
    wih1m = nc.dram_tensor("wih1m", [2, 2, 4, H, H], BF16, kind="ExternalInput")
    whh1m = nc.dram_tensor("whh1m", [2, 4, H, H], BF16, kind="ExternalInput")
    b1m = nc.dram_tensor("b1m", [2, 4, 1, H], BF16, kind="ExternalInput")
    woutm = nc.dram_tensor("woutm", [2, H, K], BF16, kind="ExternalInput")
    boutv = nc.dram_tensor("boutv", [K, 1], F32, kind="ExternalInput")
    transm = nc.dram_tensor("transm", [K, K], F32, kind="ExternalInput")
    transTm = nc.dram_tensor("transTm", [K, K], F32, kind="ExternalInput")
    transbm = nc.dram_tensor("transbm", [K, K], BF16, kind="ExternalInput")
    startv = nc.dram_tensor("startv", [K, 1], F32, kind="ExternalInput")
    endv = nc.dram_tensor("endv", [K, 1], F32, kind="ExternalInput")
    outm = nc.dram_tensor("outm", [2, BL], F32, kind="ExternalOutput")

    with tile.TileContext(nc) as tc, contextlib.ExitStack() as ctx:
        big = ctx.enter_context(tc.tile_pool(name="big", bufs=1))
        wp = ctx.enter_context(tc.tile_pool(name="wp", bufs=1))
        work = ctx.enter_context(tc.tile_pool(name="work", bufs=3))
        gw = ctx.enter_context(tc.tile_pool(name="gw", bufs=2))
        stp = ctx.enter_context(tc.tile_pool(name="stp", bufs=2))

        # ---------------- P0: constants, weights, gather ----------------
        idx = wp.tile([128, nt], I16, tag="idx")
        nc.gpsimd.memset(idx[:], 0)
        nc.sync.dma_start(out=idx[0:BL, :], in_=toks16[:, :])

        def load_w(name, dram_ap, shape, dt=BF16):
            t = wp.tile(shape, dt, tag=name)
            nc.sync.dma_start(out=t[:], in_=dram_ap)
            return t

        wih0 = [[load_w(f"wih0_{d}{g}", wih0m[d, g], [E, H]) for g in range(4)]
                for d in range(2)]
        whh0 = [[load_w(f"whh0_{d}{g}", whh0m[d, g], [H, H]) for g in range(4)]
                for d in range(2)]
        b0 = [[load_w(f"b0_{d}{g}", b0m[d, g], [1, H]) for g in range(4)]
              for d in range(2)]
        wih1 = [[[load_w(f"wih1_{d}{s}{g}", wih1m[d, s, g], [H, H])
                  for g in range(4)] for s in range(2)] for d in range(2)]
        whh1 = [[load_w(f"whh1_{d}{g}", whh1m[d, g], [H, H]) for g in range(4)]
                for d in range(2)]
        b1 = [[load_w(f"b1_{d}{g}", b1m[d, g], [1, H]) for g in range(4)]
              for d in range(2)]
        wout_sb = [load_w(f"wout_{d}", woutm[d], [H, K]) for d in range(2)]
        bout_sb = load_w("bout", boutv[:, :], [K, 1], F32)
        trans_sb = load_w("trans", transm[:, :], [K, K], F32)
        transT_sb = load_w("transT", transTm[:, :], [K, K], F32)
        transb_sb = load_w("transb", transbm[:, :], [K, K], BF16)
        start_sb = load_w("start", startv[:, :], [K, 1], F32)
        end_sb = load_w("end", endv[:, :], [K, 1], F32)

        onesb = wp.tile([1, GRP * BL], BF16, tag="onesb")
        nc.vector.memset(onesb[:], 1.0)
        ones20 = wp.tile([K, 1], F32, tag="ones20")
        nc.vector.memset(ones20[:], 1.0)
        ones2020 = wp.tile([K, K], F32, tag="ones2020")
        nc.vector.memset(ones2020[:], 1.0)
        zeros16 = wp.tile([128, BL], BF16, tag="zeros16")
        nc.vector.memset(zeros16[:], 0.0)
        iota20 = wp.tile([K, 1], mybir.dt.int32, tag="iota20i")
        nc.gpsimd.iota(iota20[:], pattern=[[0, 1]], base=0,
                       channel_multiplier=1)
        iota20f = wp.tile([K, 1], F32, tag="iota20f")
        nc.vector.tensor_copy(iota20f[:], iota20[:])
        eexp = wp.tile([K, K], F32, tag="eexp")
        nc.scalar.activation(eexp[:], trans_sb[:], AF.Exp)
        eexpT = wp.tile([K, K], F32, tag="eexpT")
        nc.scalar.activation(eexpT[:], transT_sb[:], AF.Exp)
        expstart = wp.tile([K, 1], F32, tag="expstart")
        nc.scalar.activation(expstart[:], start_sb[:], AF.Exp)
        expend = wp.tile([K, 1], F32, tag="expend")
        nc.scalar.activation(expend[:], end_sb[:], AF.Exp)

        # Embedding gather: xT [E=128, NTB] bf16, col = t*BL + b (t-major)
        xT = big.tile([128, NTB], BF16, tag="xT")
        xT3 = xT.rearrange("p (o c) -> p o c", o=1)
        GCH = 256
        ngch = max(1, NTB // GCH)
        # interleave from both ends so fwd (head) and bwd (tail) scans can
        # start before the full gather completes
        order = []
        for i in range((ngch + 1) // 2):
            order.append(ngch - 1 - i)
            if i != ngch - 1 - i:
                order.append(i)
        for g in order:
            cw = min(GCH, NTB)
            nc.gpsimd.dma_gather(
                xT3[:, :, g * cw:(g + 1) * cw], embedb[:, :],
                idx[:, g * (cw // BL):(g + 1) * (cw // BL)],
                cw, cw, E, transpose=True)

        # tags -> one-hot OH [K, NTB] bf16 (built in chunks, dripped)
        tags_rep = big.tile([K, NTB], F32, tag="expem", name="tags_rep")
        nc.sync.dma_start(out=tags_rep[:],
                          in_=tagsf[0:1, :].to_broadcast([K, NTB]))
        OHt = big.tile([K, NTB], BF16, tag="OH")

        drip = []   # queue of zero-arg emit callbacks, popped ~2/step

        CHW = CT * BL  # 512 cols per chunk
        for c in range(NCH):
            def mk_oh(c=c):
                sl = slice(c * CHW, (c + 1) * CHW)
                nc.vector.tensor_tensor(
                    OHt[:, sl], iota20f[:, 0:1].to_broadcast([K, CHW]),
                    tags_rep[:, sl], OP.is_equal)
            drip.append(mk_oh)

        # Histories (bf16, feature-major, t-major cols)
        h0T = [big.tile([H, NTB], BF16, tag=f"h0T{d}", name=f"h0T{d}")
               for d in range(2)]
        h1T = [big.tile([H, NTB], BF16, tag=f"h1T{d}", name=f"h1T{d}")
               for d in range(2)]
        expem = big.tile([K, NTB], F32, tag="expem", name="expem")
        emfull = big.tile([K, NTB], F32, tag="em")
        nacc = wp.tile([K, max(1, NCH) * BL], F32, tag="nacc")

        # ---------------- the scan ----------------
        def scan_layer(layer, srcs, WIH, WHH, BB, hist_out, extra_cb=None):
            """srcs: list of full-width [128, NTB] moving tiles (1 or 2).
            WIH[d][s][g], WHH[d][g], BB[d][g]."""
            with tc.tile_pool(name=f"zp{layer}0", bufs=2, space="PSUM") as zf, \
                 tc.tile_pool(name=f"zp{layer}1", bufs=2, space="PSUM") as zb:
                zp = [zf, zb]
                banks = [None, None]
                cprev = []
                for d in range(2):
                    cp0 = stp.tile([H, BL], F32, tag=f"c{layer}{d}")
                    nc.vector.memset(cp0[:], 0.0)
                    cprev.append(cp0)

                def precompute(d, k):
                    bank = zp[d].tile([128, GRP * 64], F32, tag=f"z{d}")
                    t0 = k * GRP if d == 0 else nt - (k + 1) * GRP
                    cols = slice(t0 * BL, (t0 + GRP) * BL)
                    b3 = bank.rearrange("p (t c) -> p t c", c=64)
                    for si in range(len(srcs)):
                        for g in range(4):
                            nc.tensor.matmul(
                                b3[:, :, g * BL:(g + 1) * BL],
                                WIH[d][si][g][:], srcs[si][:, cols],
                                start=(si == 0), stop=False,
                                skip_group_check=True)
                    for g in range(4):
                        nc.tensor.matmul(
                            b3[:, :, g * BL:(g + 1) * BL],
                            BB[d][g][:], onesb[:, :],
                            start=False, stop=False, skip_group_check=True)
                    return bank

                banks[0] = precompute(0, 0)
                banks[1] = precompute(1, 0)
                nbanks = [None, None]
                for n in range(nt):
                    if n % GRP == 0 and n + GRP < nt:
                        nbanks[0] = precompute(0, n // GRP + 1)
                        nbanks[1] = precompute(1, n // GRP + 1)
                    tt_ = [n, nt - 1 - n]
                    toffs = [n % GRP, GRP - 1 - (n % GRP)]
                    # 1) recurrent matmuls (both dirs)
                    import os as _os
                    for d in range(2):
                        t_ = tt_[d]
                        bank = banks[d]
                        if n == 0 or _os.environ.get("KK_BREAK_H"):
                            hprev = zeros16[:, :]
                        else:
                            tp = t_ + (-1 if d == 0 else 1)
                            hprev = hist_out[d][:, tp * BL:(tp + 1) * BL]
                        for g in range(4):
                            o = toffs[d] * 64 + g * BL
                            nc.tensor.matmul(
                                bank[:, o:o + BL], WHH[d][g][:], hprev,
                                start=False, stop=True,
                                skip_group_check=True)
                    # 2) gate sigmoid (all 4 gates, one instr per dir)
                    ss = []
                    for d in range(2):
                        s = work.tile([H, 64], F32, tag=f"s{d}")
                        o = toffs[d] * 64
                        nc.scalar.activation(s[:], banks[d][:, o:o + 64],
                                             AF.Sigmoid)
                        ss.append(s)
                    # 3-5) per-chain grouped: u, fc, c~
                    VAR = _os.environ.get("KK_CVAR", "B")
                    us, fcs = [], []
                    for d in range(2):
                        u = work.tile([H, BL], F32, tag=f"u{d}")
                        nc.vector.scalar_tensor_tensor(
                            u[:], ss[d][:, 2 * BL:3 * BL], -0.5,
                            ss[d][:, 0:BL], OP.add, OP.mult)
                        us.append(u)
                        fc = work.tile([H, BL], F32, tag=f"fc{d}")
                        fc_eng = nc.vector if VAR == "B" else nc.gpsimd
                        fc_eng.tensor_tensor(
                            fc[:], ss[d][:, BL:2 * BL], cprev[d][:], OP.mult)
                        fcs.append(fc)
                        cn = stp.tile([H, BL], F32, tag=f"c{layer}{d}")
                        u2 = fc if _os.environ.get("KK_SKIP_U") else u
                        c_eng = nc.gpsimd if VAR == "C" else nc.vector
                        c_eng.tensor_tensor(cn[:], fc[:], u2[:], OP.add)
                        cprev[d] = cn
                    # 6) sc = sigmoid(4 c~)
                    scs = []
                    for d in range(2):
                        sc = work.tile([H, BL], F32, tag=f"sc{d}")
                        src = cp0 if _os.environ.get("KK_SKIP_SC") \
                            else cprev[d]
                        nc.scalar.activation(sc[:], src[:], AF.Sigmoid,
                                             scale=4.0)
                        scs.append(sc)
                    # 7) h~ = (sc - .5) * so  -> bf16 hist (read by next mm)
                    for d in range(2):
                        t_ = tt_[d]
                        nc.vector.scalar_tensor_tensor(
                            hist_out[d][:, t_ * BL:(t_ + 1) * BL],
                            scs[d][:], -0.5, ss[d][:, 3 * BL:4 * BL],
                            OP.add, OP.mult)
                    if n % GRP == GRP - 1:
                        banks[0], banks[1] = nbanks[0], nbanks[1]
                    if extra_cb is not None:
                        extra_cb(n)
                    for _ in range(2):
                        if drip:
                            drip.pop(0)()

        scan_layer(0, [xT], [[wih0[d]] for d in range(2)], whh0, b0, h0T)

        # ------- emissions + numerator chunks, dripped into L1 -------
        em_ctx = contextlib.ExitStack()
        ep = em_ctx.enter_context(tc.tile_pool(name="ep", bufs=2, space="PSUM"))
        sp1 = em_ctx.enter_context(tc.tile_pool(name="sp1", bufs=2,
                                                space="PSUM"))

        def emit_chunk_ops(c):
            """Ops for emissions chunk c (cols [c*CHW, (c+1)*CHW))."""
            sl = slice(c * CHW, (c + 1) * CHW)
            ops = []
            pe_box = {}

            def op_mm():
                pe = ep.tile([K, CHW], F32, tag="pe")
                nc.tensor.matmul(pe[:], wout_sb[0][:], h1T[0][:, sl],
                                 start=True, stop=False)
                nc.tensor.matmul(pe[:], wout_sb[1][:], h1T[1][:, sl],
                                 start=False, stop=True)
                pe_box["pe"] = pe
            ops.append(op_mm)

            def op_g0():
                # em (+bout) -> persistent SBUF; Identity shares the Sigmoid
                # act table so no table reload mid-scan. Exp deferred to one
                # big post-scan instruction.
                nc.scalar.activation(emfull[:, sl], pe_box["pe"][:],
                                     AF.Identity, bias=bout_sb[:, 0:1])
            ops.append(op_g0)

            def op_s1():
                # S1[j, col] = trans[tag_{col-BL}, j] for col >= BL
                lo = c * CHW
                s1 = sp1.tile([K, CHW], F32, tag="s1")
                if c == 0:
                    nc.tensor.matmul(s1[:, BL:], transb_sb[:],
                                     OHt[:, 0:CHW - BL], start=True, stop=True)
                else:
                    nc.tensor.matmul(s1[:], transb_sb[:],
                                     OHt[:, lo - BL:lo + CHW - BL],
                                     start=True, stop=True)
                pe_box["s1"] = s1
            ops.append(op_s1)

            HW2 = CHW // 2

            def op_gf1():
                gf = gw.tile([K, CHW], F32, tag="gf")
                g0, s1 = emfull[:, sl], pe_box["s1"]
                if c == 0:
                    # t=0: G = em + start (no trans term)
                    nc.gpsimd.tensor_tensor(
                        gf[:, 0:BL], g0[:, 0:BL],
                        start_sb[:, 0:1].to_broadcast([K, BL]), OP.add)
                    nc.vector.tensor_tensor(gf[:, BL:HW2], g0[:, BL:HW2],
                                            s1[:, BL:HW2], OP.add)
                else:
                    nc.vector.tensor_tensor(gf[:, 0:HW2], g0[:, 0:HW2],
                                            s1[:, 0:HW2], OP.add)
                pe_box["gf"] = gf
            ops.append(op_gf1)

            def op_gf2():
                gf, s1 = pe_box["gf"], pe_box["s1"]
                g0 = emfull[:, sl]
                nc.vector.tensor_tensor(gf[:, HW2:], g0[:, HW2:],
                                        s1[:, HW2:], OP.add)
            ops.append(op_gf2)

            for b in range(BL):
                def op_nb(b=b):
                    gf = pe_box["gf"]
                    g3 = gf.rearrange("p (t b) -> p t b", b=BL)
                    o3 = OHt[:, sl].rearrange("p (t b) -> p t b", b=BL)
                    dump = gw.tile([K, CT], F32, tag="dump")
                    d3 = dump.rearrange("p (t o) -> p t o", o=1)
                    nc.vector.scalar_tensor_tensor(
                        d3[:], g3[:, :, b:b + 1], 0.0, o3[:, :, b:b + 1],
                        OP.add, OP.mult,
                        accum_out=nacc[:, c * BL + b:c * BL + b + 1])
                ops.append(op_nb)
            return ops

        # chunk readiness: step max(CT*(c+1)-1, nt-1-CT*c)
        import os as _os
        pend = {}
        for c in range(NCH):
            if _os.environ.get("KK_EMIT_LATE"):
                rc = nt - 1
            else:
                rc = max(CT * (c + 1) - 1, nt - 1 - CT * c)
            pend.setdefault(rc, []).extend(emit_chunk_ops(c))

        def l1_cb(n):
            if n in pend:
                drip.extend(pend.pop(n))

        scan_layer(1, [h0T[0], h0T[1]], wih1, whh1, b1, h1T, extra_cb=l1_cb)
        for f in drip:
            f()
        drip.clear()
        em_ctx.close()
        # one Exp for the whole emissions tensor (single act-table switch)
        nc.scalar.activation(expem[:], emfull[:], AF.Exp)

        # ---------------- CRF: two concurrent chains ----------------
        TM = nt // 2 - 1   # alpha meets v at t=TM
        with tc.tile_pool(name="cp", bufs=2, space="PSUM") as cp, \
             tc.tile_pool(name="sp", bufs=1, space="PSUM") as sp:
            a = stp.tile([K, BL], F32, tag="alpha")
            nc.vector.tensor_tensor(
                a[:], expem[:, 0:BL],
                expstart[:, 0:1].to_broadcast([K, BL]), OP.mult)
            uv = stp.tile([K, BL], F32, tag="uv")
            nc.vector.tensor_tensor(
                uv[:], expem[:, (nt - 1) * BL:nt * BL],
                expend[:, 0:1].to_broadcast([K, BL]), OP.mult)
            la = stp.tile([1, BL], F32, tag="lacc0")
            nc.vector.memset(la[:], 0.0)
            lv = stp.tile([1, BL], F32, tag="lacc1")
            nc.vector.memset(lv[:], 0.0)
            laccs = [la, lv]
            pendings = [None, None]

            def rescale(hh, cur, i):
                ps = sp.tile([K, BL], F32, tag=f"ps{hh}")
                nc.tensor.matmul(ps[:], ones2020[:], cur[:],
                                 start=True, stop=True)
                sinv = work.tile([K, BL], F32, tag=f"sinv{hh}")
                nc.vector.reciprocal(sinv[:], ps[:])
                lt = work.tile([1, BL], F32, tag=f"lt{hh}")
                nc.scalar.activation(lt[:], ps[0:1, :], AF.Ln)
                lnew = stp.tile([1, BL], F32, tag=f"lacc{hh}")
                nc.vector.tensor_tensor(lnew[:], laccs[hh][:], lt[:], OP.add)
                laccs[hh] = lnew
                pendings[hh] = (sinv, i + 2)

            def apply_pending(hh, cur, tag):
                if pendings[hh] is not None:
                    nw = stp.tile([K, BL], F32, tag=tag)
                    nc.vector.tensor_tensor(nw[:], cur[:],
                                            pendings[hh][0][:], OP.mult)
                    pendings[hh] = None
                    return nw
                return cur

            nsteps = TM   # alpha steps t=1..TM ; v steps t=nt-2..TM+1
            for i in range(nsteps):
                ta = 1 + i
                tv = nt - 2 - i
                # alpha: pa = Eexp^T a ; a = pa * expem_ta
                pa = cp.tile([K, BL], F32, tag="pa")
                nc.tensor.matmul(pa[:], eexp[:], a[:], start=True, stop=True)
                # v: pv = E uv ; uv = pv * expem_tv
                pv = cp.tile([K, BL], F32, tag="pv")
                nc.tensor.matmul(pv[:], eexpT[:], uv[:], start=True, stop=True)
                an = stp.tile([K, BL], F32, tag="alpha")
                nc.vector.tensor_tensor(an[:], pa[:],
                                        expem[:, ta * BL:(ta + 1) * BL],
                                        OP.mult)
                a = an
                un = stp.tile([K, BL], F32, tag="uv")
                nc.vector.tensor_tensor(un[:], pv[:],
                                        expem[:, tv * BL:(tv + 1) * BL],
                                        OP.mult)
                uv = un
                if pendings[0] is not None and i >= pendings[0][1]:
                    a = apply_pending(0, a, "alpha")
                if pendings[1] is not None and i >= pendings[1][1]:
                    uv = apply_pending(1, uv, "uv")
                if i % RESCALE == RESCALE - 1 and i + 3 < nsteps:
                    rescale(0, a, i)
                    rescale(1, uv, i)
            a = apply_pending(0, a, "alpha")
            uv = apply_pending(1, uv, "uv")
            # final v_{TM} = E uv_{TM+1} ; Z = <a, v_TM>
            pv = cp.tile([K, BL], F32, tag="pv")
            nc.tensor.matmul(pv[:], eexpT[:], uv[:], start=True, stop=True)
            q = work.tile([K, BL], F32, tag="q")
            nc.vector.tensor_tensor(q[:], a[:], pv[:], OP.mult)
            psz = sp.tile([1, BL], F32, tag="psz")
            nc.tensor.matmul(psz[:], ones20[:], q[:], start=True, stop=True)
            lnz = work.tile([1, BL], F32, tag="lnz")
            nc.scalar.activation(lnz[:], psz[:], AF.Ln)
            logz = work.tile([1, BL], F32, tag="logz")
            nc.vector.tensor_tensor(logz[:], lnz[:], laccs[0][:], OP.add)
            logz2 = work.tile([1, BL], F32, tag="logz2")
            nc.vector.tensor_tensor(logz2[:], logz[:], laccs[1][:], OP.add)
            nc.sync.dma_start(out=outm[1:2, :], in_=logz2[:])

            # ---- numerator finish ----
            # reduce nacc [K, NCH*BL] (chunk-major) -> [K, BL]
            cur = nacc
            width = NCH * BL
            while width > BL:
                half = width // 2
                red = work.tile([K, half], F32, tag="red")
                nc.vector.tensor_tensor(red[:], cur[:, 0:half],
                                        cur[:, half:width], OP.add)
                cur = red
                width = half
            qe = work.tile([K, BL], F32, tag="qe")
            nc.vector.tensor_tensor(
                qe[:], OHt[:, (nt - 1) * BL:nt * BL],
                end_sb[:, 0:1].to_broadcast([K, BL]), OP.mult)
            stot = work.tile([K, BL], F32, tag="stot")
            nc.vector.tensor_tensor(stot[:], cur[:, 0:BL], qe[:], OP.add)
            pss = sp.tile([1, BL], F32, tag="pss")
            nc.tensor.matmul(pss[:], ones20[:], stot[:], start=True, stop=True)
            score = work.tile([1, BL], F32, tag="score")
            nc.vector.tensor_copy(score[:], pss[:])
            nc.sync.dma_start(out=outm[0:1, :], in_=score[:])

    nc.compile()
    return nc


# ---------------------------------------------------------------------------
# Host side
# ---------------------------------------------------------------------------
_CACHE = {}


def _get_nc(nt):
    if nt not in _CACHE:
        _CACHE[nt] = build(nt)
    return _CACHE[nt]


def prep_inputs(sentences, tags, embed, Wih0, Whh0, b0, Wih1, Whh1, b1,
                Wout, bout, trans, start, end, nt=T):
    """Host-side marshalling: gate-split transposed bf16 weights."""
    f32 = np.float32
    bf16 = ml_dtypes.bfloat16
    gs = np.array([1.0, 1.0, 2.0, 1.0], f32)  # g-gate rows x2 (tanh trick)

    def wT(w, extra):
        # w [4H, D] -> [4, D, H], rows scaled by gs*extra, transposed
        out = np.empty((4, w.shape[1], H), f32)
        for g in range(4):
            out[g] = (w[g * H:(g + 1) * H] * (gs[g] * extra)).T
        return out

    wih0 = np.stack([wT(Wih0[d], 1.0) for d in range(2)]).astype(bf16)
    whh0 = np.stack([wT(Whh0[d], 2.0) for d in range(2)]).astype(bf16)
    b0v = np.stack([(b0[d].reshape(4, H) * gs[:, None]).reshape(4, 1, H)
                    for d in range(2)]).astype(bf16)
    wih1f = np.stack([wT(Wih1[d], 2.0) for d in range(2)])  # [2,4,2H,H]
    wih1 = np.ascontiguousarray(
        wih1f.reshape(2, 4, 2, H, H).transpose(0, 2, 1, 3, 4)).astype(bf16)
    whh1 = np.stack([wT(Whh1[d], 2.0) for d in range(2)]).astype(bf16)
    b1v = np.stack([(b1[d].reshape(4, H) * gs[:, None]).reshape(4, 1, H)
                    for d in range(2)]).astype(bf16)
    woutT = np.stack([np.ascontiguousarray((2.0 * Wout[:, :H]).T),
                      np.ascontiguousarray((2.0 * Wout[:, H:]).T)]).astype(bf16)
    shared = dict(
        embedb=np.ascontiguousarray(embed.astype(bf16)),
        wih0m=wih0, whh0m=whh0, b0m=b0v, wih1m=wih1, whh1m=whh1, b1m=b1v,
        woutm=woutT, boutv=bout.reshape(K, 1).astype(f32),
        transm=trans.astype(f32),
        transTm=np.ascontiguousarray(trans.T).astype(f32),
        transbm=trans.astype(bf16),
        startv=start.reshape(K, 1).astype(f32),
        endv=end.reshape(K, 1).astype(f32),
    )
    in_maps = []
    for c in range(NCORES):
        bsl = slice(c * BL, (c + 1) * BL)
        m = dict(shared)
        m["toks16"] = np.ascontiguousarray(
            sentences[bsl, :nt].astype(np.int16))
        m["tagsf"] = np.ascontiguousarray(
            tags[bsl, :nt].T.astype(f32).reshape(1, BL * nt))  # t-major
        in_maps.append(m)
    return in_maps


def run(inputs_np, nt=T, trace=False):
    nc = _get_nc(nt)
    in_maps = prep_inputs(
        inputs_np["sentences"], inputs_np["tags"], inputs_np["embed"],
        inputs_np["Wih0"], inputs_np["Whh0"], inputs_np["b0"],
        inputs_np["Wih1"], inputs_np["Whh1"], inputs_np["b1"],
        inputs_np["Wout"], inputs_np["bout"], inputs_np["trans"],
        inputs_np["start"], inputs_np["end"], nt=nt)
    res = run_bass_kernel_spmd(nc, in_maps, core_ids=list(range(NCORES)),
                               trace=trace)
    score = np.concatenate([res.results[c]["outm"][0] for c in range(NCORES)])
    logz = np.concatenate([res.results[c]["outm"][1] for c in range(NCORES)])
    loss = -np.mean(score - logz)
    return np.float32(loss), res


def kernel(**inputs):
    inputs_np = {k: np.asarray(v) for k, v in inputs.items()}
    loss, _ = run(inputs_np, nt=T)
    return np.asarray(loss, dtype=np.float32)


# revision 3
# speedup vs baseline: 1.8896x; 1.0000x over previous
"""BiLSTM-CRF loss kernel v2 for 8x Trainium2 NeuronCores (Bass/Tile).

Data-parallel over batch (16 sentences/core), feature-major layout:
hidden dim H=128 on partitions, batch in the free dim.

Per LSTM tick per direction the serial chain is only:
  4 tiny rec matmuls (bf16, 16 cols) -> sigmoid [128,64] (all 4 gates)
  -> u/fc/c~ (DVE+Pool) -> sigmoid(4c~) [128,16] -> h~ stt -> next matmul.
Input projections x@Wih + b are pre-accumulated into the same PSUM banks
8 ticks at a time by wide matmuls, so no per-tick add is needed.

Math identical to baseline: tanh via sigmoid (g-rows x2 in weights),
h~ = h/2 (x2 folded into Whh/Wih1/Wout), c~ = c/2, sc = sigmoid(4*c~).

CRF: exp-space with periodic rescaling, split into TWO concurrent chains
meeting in the middle:  Z = alpha_{M} . v_{M},  alpha forward from t=0,
v backward from t=T-1 (v_t = E @ (v_{t+1} * expem_{t+1})).

Numerator: one-hot tags OH [K,NTB]; G = Wout.h1+bout+trans^T@OH(shift)
accumulated in PSUM; score_b = sum_t <G(:,t,b), OH(:,t,b)> + end-term.
Emissions/numerator work drips into the layer-1 scan as chunks of h1
become ready.
"""

import sys

sys.path.insert(0, "/opt/trn_rl_repo")

import contextlib

import numpy as np
import ml_dtypes

import concourse.bass as bass
import concourse.tile as tile
from concourse import bacc, mybir
from concourse.bass_utils import run_bass_kernel_spmd

F32 = mybir.dt.float32
BF16 = mybir.dt.bfloat16
I16 = mybir.dt.int16
AF = mybir.ActivationFunctionType
OP = mybir.AluOpType

NCORES = 8
B, T, E, H, K, V = 128, 512, 128, 128, 20, 30000
BL = B // NCORES        # 16 sentences per core
GRP = 8                 # ticks per PSUM z-bank
CT = 32                 # ticks per emissions chunk (512 cols)
RESCALE = 8


def build(nt=T):
    nc = bacc.Bacc("TRN2", target_bir_lowering=False, debug=False,
                   num_devices=NCORES)
    NTB = nt * BL
    NG = nt // GRP
    NCH = NTB // (CT * BL)          # emissions chunks

    # ---- DRAM I/O ----
    embedb = nc.dram_tensor("embedb", [V, E], BF16, kind="ExternalInput")
    toks16 = nc.dram_tensor("toks16", [BL, nt], I16, kind="ExternalInput")
    tagsf = nc.dram_tensor("tagsf", [1, NTB], F32, kind="ExternalInput")  # t-major
    wih0m = nc.dram_tensor("wih0m", [2, 4, E, H], BF16, kind="ExternalInput")
    whh0m = nc.dram_tensor("whh0m", [2, 4, H, H], BF16, kind="ExternalInput")
    b0m = nc.dram_tensor("b0m", [2, 4, 1, H], BF16, kind="ExternalInput")
    wih1m = nc.dram_tensor("wih1m", [2, 2, 4, H, H], BF16, kind="ExternalInput")
    whh1m = nc.dram_tensor("whh1m", [2, 4, H, H], BF16, kind="ExternalInput")
    b1m = nc.dram_tensor("b1m", [2, 4, 1, H], BF16, kind="ExternalInput")
    woutm = nc.dram_tensor("woutm", [2, H, K], BF16, kind="ExternalInput")
    boutv = nc.dram_tensor("boutv", [K, 1], F32, kind="ExternalInput")
    transm = nc.dram_tensor("transm", [K, K], F32, kind="ExternalInput")
    transTm = nc.dram_tensor("transTm", [K, K], F32, kind="ExternalInput")
    transbm = nc.dram_tensor("transbm", [K, K], BF16, kind="ExternalInput")
    startv = nc.dram_tensor("startv", [K, 1], F32, kind="ExternalInput")
    endv = nc.dram_tensor("endv", [K, 1], F32, kind="ExternalInput")
    outm = nc.dram_tensor("outm", [2, BL], F32, kind="ExternalOutput")

    with tile.TileContext(nc) as tc, contextlib.ExitStack() as ctx:
        big = ctx.enter_context(tc.tile_pool(name="big", bufs=1))
        wp = ctx.enter_context(tc.tile_pool(name="wp", bufs=1))
        work = ctx.enter_context(tc.tile_pool(name="work", bufs=4))
        gw = ctx.enter_context(tc.tile_pool(name="gw", bufs=2))
        stp = ctx.enter_context(tc.tile_pool(name="stp", bufs=3))

        # ---------------- P0: constants, weights, gather ----------------
        idx = wp.tile([128, nt], I16, tag="idx")
        nc.gpsimd.memset(idx[:], 0)
        nc.sync.dma_start(out=idx[0:BL, :], in_=toks16[:, :])

        def load_w(name, dram_ap, shape, dt=BF16):
            t = wp.tile(shape, dt, tag=name)
            nc.sync.dma_start(out=t[:], in_=dram_ap)
            return t

        wih0 = [[load_w(f"wih0_{d}{g}", wih0m[d, g], [E, H]) for g in range(4)]
                for d in range(2)]
        whh0 = [[load_w(f"whh0_{d}{g}", whh0m[d, g], [H, H]) for g in range(4)]
                for d in range(2)]
        b0 = [[load_w(f"b0_{d}{g}", b0m[d, g], [1, H]) for g in range(4)]
              for d in range(2)]
        wih1 = [[[load_w(f"wih1_{d}{s}{g}", wih1m[d, s, g], [H, H])
                  for g in range(4)] for s in range(2)] for d in range(2)]
        whh1 = [[load_w(f"whh1_{d}{g}", whh1m[d, g], [H, H]) for g in range(4)]
                for d in range(2)]
        b1 = [[load_w(f"b1_{d}{g}", b1m[d, g], [1, H]) for g in range(4)]
              for d in range(2)]
        wout_sb = [load_w(f"wout_{d}", woutm[d], [H, K]) for d in range(2)]
        bout_sb = load_w("bout", boutv[:, :], [K, 1], F32)
        trans_sb = load_w("trans", transm[:, :], [K, K], F32)
        transT_sb = load_w("transT", transTm[:, :], [K, K], F32)
        transb_sb = load_w("transb", transbm[:, :], [K, K], BF16)
        start_sb = load_w("start", startv[:, :], [K, 1], F32)
        end_sb = load_w("end", endv[:, :], [K, 1], F32)

        onesb = wp.tile([1, GRP * BL], BF16, tag="onesb")
        nc.vector.memset(onesb[:], 1.0)
        ones20 = wp.tile([K, 1], F32, tag="ones20")
        nc.vector.memset(ones20[:], 1.0)
        ones2020 = wp.tile([K, K], F32, tag="ones2020")
        nc.vector.memset(ones2020[:], 1.0)
        zeros16 = wp.tile([128, BL], BF16, tag="zeros16")
        nc.vector.memset(zeros16[:], 0.0)
        iota20 = wp.tile([K, 1], mybir.dt.int32, tag="iota20i")
        nc.gpsimd.iota(iota20[:], pattern=[[0, 1]], base=0,
                       channel_multiplier=1)
        iota20f = wp.tile([K, 1], F32, tag="iota20f")
        nc.vector.tensor_copy(iota20f[:], iota20[:])
        eexp = wp.tile([K, K], F32, tag="eexp")
        nc.scalar.activation(eexp[:], trans_sb[:], AF.Exp)
        eexpT = wp.tile([K, K], F32, tag="eexpT")
        nc.scalar.activation(eexpT[:], transT_sb[:], AF.Exp)
        expstart = wp.tile([K, 1], F32, tag="expstart")
        nc.scalar.activation(expstart[:], start_sb[:], AF.Exp)
        expend = wp.tile([K, 1], F32, tag="expend")
        nc.scalar.activation(expend[:], end_sb[:], AF.Exp)

        # Embedding gather: xT [E=128, NTB] bf16, col = t*BL + b (t-major)
        xT = big.tile([128, NTB], BF16, tag="xT")
        xT3 = xT.rearrange("p (o c) -> p o c", o=1)
        GCH = 256
        ngch = max(1, NTB // GCH)
        # interleave from both ends so fwd (head) and bwd (tail) scans can
        # start before the full gather completes
        order = []
        for i in range((ngch + 1) // 2):
            order.append(ngch - 1 - i)
            if i != ngch - 1 - i:
                order.append(i)
        for g in order:
            cw = min(GCH, NTB)
            nc.gpsimd.dma_gather(
                xT3[:, :, g * cw:(g + 1) * cw], embedb[:, :],
                idx[:, g * (cw // BL):(g + 1) * (cw // BL)],
                cw, cw, E, transpose=True)

        # tags -> one-hot OH [K, NTB] bf16 (built in chunks, dripped)
        tags_rep = big.tile([K, NTB], F32, tag="expem", name="tags_rep")
        nc.sync.dma_start(out=tags_rep[:],
                          in_=tagsf[0:1, :].to_broadcast([K, NTB]))
        OHt = big.tile([K, NTB], BF16, tag="OH")

        drip = []   # queue of zero-arg emit callbacks, popped ~2/step

        CHW = CT * BL  # 512 cols per chunk
        for c in range(NCH):
            def mk_oh(c=c):
                sl = slice(c * CHW, (c + 1) * CHW)
                nc.vector.tensor_tensor(
                    OHt[:, sl], iota20f[:, 0:1].to_broadcast([K, CHW]),
                    tags_rep[:, sl], OP.is_equal)
            drip.append(mk_oh)

        # Histories (bf16, feature-major, t-major cols)
        h0T = [big.tile([H, NTB], BF16, tag=f"h0T{d}", name=f"h0T{d}")
               for d in range(2)]
        h1T = [big.tile([H, NTB], BF16, tag=f"h1T{d}", name=f"h1T{d}")
               for d in range(2)]
        expem = big.tile([K, NTB], F32, tag="expem", name="expem")
        emfull = big.tile([K, NTB], F32, tag="em")
        nacc = wp.tile([K, max(1, NCH) * BL], F32, tag="nacc")

        # ---------------- the scan ----------------
        def scan_layer(layer, srcs, WIH, WHH, BB, hist_out, extra_cb=None):
            """srcs: list of full-width [128, NTB] moving tiles (1 or 2).
            WIH[d][s][g], WHH[d][g], BB[d][g]."""
            with tc.tile_pool(name=f"zp{layer}0", bufs=2, space="PSUM") as zf, \
                 tc.tile_pool(name=f"zp{layer}1", bufs=2, space="PSUM") as zb:
                zp = [zf, zb]
                banks = [None, None]
                cprev = []
                for d in range(2):
                    cp0 = stp.tile([H, BL], F32, tag=f"c{layer}{d}")
                    nc.vector.memset(cp0[:], 0.0)
                    cprev.append(cp0)

                def precompute(d, k):
                    bank = zp[d].tile([128, GRP * 64], F32, tag=f"z{d}")
                    t0 = k * GRP if d == 0 else nt - (k + 1) * GRP
                    cols = slice(t0 * BL, (t0 + GRP) * BL)
                    b3 = bank.rearrange("p (t c) -> p t c", c=64)
                    for si in range(len(srcs)):
                        for g in range(4):
                            nc.tensor.matmul(
                                b3[:, :, g * BL:(g + 1) * BL],
                                WIH[d][si][g][:], srcs[si][:, cols],
                                start=(si == 0), stop=False,
                                skip_group_check=True)
                    for g in range(4):
                        nc.tensor.matmul(
                            b3[:, :, g * BL:(g + 1) * BL],
                            BB[d][g][:], onesb[:, :],
                            start=False, stop=False, skip_group_check=True)
                    return bank

                banks[0] = precompute(0, 0)
                banks[1] = precompute(1, 0)
                nbanks = [None, None]
                for n in range(nt):
                    if n % GRP == 0 and n + GRP < nt:
                        nbanks[0] = precompute(0, n // GRP + 1)
                        nbanks[1] = precompute(1, n // GRP + 1)
                    tt_ = [n, nt - 1 - n]
                    toffs = [n % GRP, GRP - 1 - (n % GRP)]
                    # 1) recurrent matmuls (both dirs)
                    import os as _os
                    for d in range(2):
                        t_ = tt_[d]
                        bank = banks[d]
                        if n == 0 or _os.environ.get("KK_BREAK_H"):
                            hprev = zeros16[:, :]
                        else:
                            tp = t_ + (-1 if d == 0 else 1)
                            hprev = hist_out[d][:, tp * BL:(tp + 1) * BL]
                        for g in range(4):
                            o = toffs[d] * 64 + g * BL
                            nc.tensor.matmul(
                                bank[:, o:o + BL], WHH[d][g][:], hprev,
                                start=False, stop=True,
                                skip_group_check=True)
                    # 2) gate sigmoid (all 4 gates, one instr per dir)
                    ss = []
                    for d in range(2):
                        s = work.tile([H, 64], F32, tag=f"s{d}")
                        o = toffs[d] * 64
                        nc.scalar.activation(s[:], banks[d][:, o:o + 64],
                                             AF.Sigmoid)
                        ss.append(s)
                    # 3-5) per-chain grouped: u, fc, c~
                    VAR = _os.environ.get("KK_CVAR", "B")
                    us, fcs = [], []
                    for d in range(2):
                        u = work.tile([H, BL], F32, tag=f"u{d}")
                        nc.vector.scalar_tensor_tensor(
                            u[:], ss[d][:, 2 * BL:3 * BL], -0.5,
                            ss[d][:, 0:BL], OP.add, OP.mult)
                        us.append(u)
                        fc = work.tile([H, BL], F32, tag=f"fc{d}")
                        fc_eng = nc.vector if VAR == "B" else nc.gpsimd
                        fc_eng.tensor_tensor(
                            fc[:], ss[d][:, BL:2 * BL], cprev[d][:], OP.mult)
                        fcs.append(fc)
                        cn = stp.tile([H, BL], F32, tag=f"c{layer}{d}")
                        u2 = fc if _os.environ.get("KK_SKIP_U") else u
                        c_eng = nc.gpsimd if VAR == "C" else nc.vector
                        c_eng.tensor_tensor(cn[:], fc[:], u2[:], OP.add)
                        cprev[d] = cn
                    # 6) sc = sigmoid(4 c~)
                    scs = []
                    for d in range(2):
                        sc = work.tile([H, BL], F32, tag=f"sc{d}")
                        src = cp0 if _os.environ.get("KK_SKIP_SC") \
                            else cprev[d]
                        nc.scalar.activation(sc[:], src[:], AF.Sigmoid,
                                             scale=4.0)
                        scs.append(sc)
                    # 7) h~ = (sc - .5) * so  -> bf16 hist (read by next mm)
                    for d in range(2):
                        t_ = tt_[d]
                        nc.vector.scalar_tensor_tensor(
                            hist_out[d][:, t_ * BL:(t_ + 1) * BL],
                            scs[d][:], -0.5, ss[d][:, 3 * BL:4 * BL],
                            OP.add, OP.mult)
                    if n % GRP == GRP - 1:
                        banks[0], banks[1] = nbanks[0], nbanks[1]
                    if extra_cb is not None:
                        extra_cb(n)
                    for _ in range(2):
                        if drip:
                            drip.pop(0)()

        scan_layer(0, [xT], [[wih0[d]] for d in range(2)], whh0, b0, h0T)

        # ------- emissions + numerator chunks, dripped into L1 -------
        em_ctx = contextlib.ExitStack()
        ep = em_ctx.enter_context(tc.tile_pool(name="ep", bufs=2, space="PSUM"))
        sp1 = em_ctx.enter_context(tc.tile_pool(name="sp1", bufs=2,
                                                space="PSUM"))

        def emit_chunk_ops(c):
            """Ops for emissions chunk c (cols [c*CHW, (c+1)*CHW))."""
            sl = slice(c * CHW, (c + 1) * CHW)
            ops = []
            pe_box = {}

            def op_mm():
                pe = ep.tile([K, CHW], F32, tag="pe")
                nc.tensor.matmul(pe[:], wout_sb[0][:], h1T[0][:, sl],
                                 start=True, stop=False)
                nc.tensor.matmul(pe[:], wout_sb[1][:], h1T[1][:, sl],
                                 start=False, stop=True)
                pe_box["pe"] = pe
            ops.append(op_mm)

            def op_g0():
                # em (+bout) -> persistent SBUF; Identity shares the Sigmoid
                # act table so no table reload mid-scan. Exp deferred to one
                # big post-scan instruction.
                nc.scalar.activation(emfull[:, sl], pe_box["pe"][:],
                                     AF.Identity, bias=bout_sb[:, 0:1])
            ops.append(op_g0)

            def op_s1():
                # S1[j, col] = trans[tag_{col-BL}, j] for col >= BL
                lo = c * CHW
                s1 = sp1.tile([K, CHW], F32, tag="s1")
                if c == 0:
                    nc.tensor.matmul(s1[:, BL:], transb_sb[:],
                                     OHt[:, 0:CHW - BL], start=True, stop=True)
                else:
                    nc.tensor.matmul(s1[:], transb_sb[:],
                                     OHt[:, lo - BL:lo + CHW - BL],
                                     start=True, stop=True)
                pe_box["s1"] = s1
            ops.append(op_s1)

            HW2 = CHW // 2

            def op_gf1():
                gf = gw.tile([K, CHW], F32, tag="gf")
                g0, s1 = emfull[:, sl], pe_box["s1"]
                if c == 0:
                    # t=0: G = em + start (no trans term)
                    nc.gpsimd.tensor_tensor(
                        gf[:, 0:BL], g0[:, 0:BL],
                        start_sb[:, 0:1].to_broadcast([K, BL]), OP.add)
                    nc.vector.tensor_tensor(gf[:, BL:HW2], g0[:, BL:HW2],
                                            s1[:, BL:HW2], OP.add)
                else:
                    nc.vector.tensor_tensor(gf[:, 0:HW2], g0[:, 0:HW2],
                                            s1[:, 0:HW2], OP.add)
                pe_box["gf"] = gf
            ops.append(op_gf1)

            def op_gf2():
                gf, s1 = pe_box["gf"], pe_box["s1"]
                g0 = emfull[:, sl]
                nc.vector.tensor_tensor(gf[:, HW2:], g0[:, HW2:],
                                        s1[:, HW2:], OP.add)
            ops.append(op_gf2)

            for b in range(BL):
                def op_nb(b=b):
                    gf = pe_box["gf"]
                    g3 = gf.rearrange("p (t b) -> p t b", b=BL)
                    o3 = OHt[:, sl].rearrange("p (t b) -> p t b", b=BL)
                    dump = gw.tile([K, CT], F32, tag="dump")
                    d3 = dump.rearrange("p (t o) -> p t o", o=1)
                    nc.vector.scalar_tensor_tensor(
                        d3[:], g3[:, :, b:b + 1], 0.0, o3[:, :, b:b + 1],
                        OP.add, OP.mult,
                        accum_out=nacc[:, c * BL + b:c * BL + b + 1])
                ops.append(op_nb)
            return ops

        # chunk readiness: step max(CT*(c+1)-1, nt-1-CT*c)
        import os as _os
        pend = {}
        for c in range(NCH):
            if _os.environ.get("KK_EMIT_LATE"):
                rc = nt - 1
            else:
                rc = max(CT * (c + 1) - 1, nt - 1 - CT * c)
            pend.setdefault(rc, []).extend(emit_chunk_ops(c))

        def l1_cb(n):
            if n in pend:
                drip.extend(pend.pop(n))

        scan_layer(1, [h0T[0], h0T[1]], wih1, whh1, b1, h1T, extra_cb=l1_cb)
        for f in drip:
            f()
        drip.clear()
        em_ctx.close()
        # one Exp for the whole emissions tensor (single act-table switch)
        nc.scalar.activation(expem[:], emfull[:], AF.Exp)

        # ---------------- CRF: two concurrent chains ----------------
        TM = nt // 2 - 1   # alpha meets v at t=TM
        with tc.tile_pool(name="cp", bufs=2, space="PSUM") as cp, \
             tc.tile_pool(name="sp", bufs=1, space="PSUM") as sp:
            a = stp.tile([K, BL], F32, tag="alpha")
            nc.vector.tensor_tensor(
                a[:], expem[:, 0:BL],
                expstart[:, 0:1].to_broadcast([K, BL]), OP.mult)
            uv = stp.tile([K, BL], F32, tag="uv")
            nc.vector.tensor_tensor(
                uv[:], expem[:, (nt - 1) * BL:nt * BL],
                expend[:, 0:1].to_broadcast([K, BL]), OP.mult)
            la = stp.tile([1, BL], F32, tag="lacc0")
            nc.vector.memset(la[:], 0.0)
            lv = stp.tile([1, BL], F32, tag="lacc1")
            nc.vector.memset(lv[:], 0.0)
            laccs = [la, lv]
            pendings = [None, None]

            def rescale(hh, cur, i):
                ps = sp.tile([K, BL], F32, tag=f"ps{hh}")
                nc.tensor.matmul(ps[:], ones2020[:], cur[:],
                                 start=True, stop=True)
                sinv = work.tile([K, BL], F32, tag=f"sinv{hh}")
                nc.vector.reciprocal(sinv[:], ps[:])
                lt = work.tile([1, BL], F32, tag=f"lt{hh}")
                nc.scalar.activation(lt[:], ps[0:1, :], AF.Ln)
                lnew = stp.tile([1, BL], F32, tag=f"lacc{hh}")
                nc.vector.tensor_tensor(lnew[:], laccs[hh][:], lt[:], OP.add)
                laccs[hh] = lnew
                pendings[hh] = (sinv, i + 2)

            def apply_pending(hh, cur, tag):
                if pendings[hh] is not None:
                    nw = stp.tile([K, BL], F32, tag=tag)
                    nc.vector.tensor_tensor(nw[:], cur[:],
                                            pendings[hh][0][:], OP.mult)
                    pendings[hh] = None
                    return nw
                return cur

            nsteps = TM   # alpha steps t=1..TM ; v steps t=nt-2..TM+1
            for i in range(nsteps):
                ta = 1 + i
                tv = nt - 2 - i
                # alpha: pa = Eexp^T a ; a = pa * expem_ta
                pa = cp.tile([K, BL], F32, tag="pa")
                nc.tensor.matmul(pa[:], eexp[:], a[:], start=True, stop=True)
                # v: pv = E uv ; uv = pv * expem_tv
                pv = cp.tile([K, BL], F32, tag="pv")
                nc.tensor.matmul(pv[:], eexpT[:], uv[:], start=True, stop=True)
                an = stp.tile([K, BL], F32, tag="alpha")
                nc.vector.tensor_tensor(an[:], pa[:],
                                        expem[:, ta * BL:(ta + 1) * BL],
                                        OP.mult)
                a = an
                un = stp.tile([K, BL], F32, tag="uv")
                nc.vector.tensor_tensor(un[:], pv[:],
                                        expem[:, tv * BL:(tv + 1) * BL],
                                        OP.mult)
                uv = un
                if pendings[0] is not None and i >= pendings[0][1]:
                    a = apply_pending(0, a, "alpha")
                if pendings[1] is not None and i >= pendings[1][1]:
                    uv = apply_pending(1, uv, "uv")
                if i % RESCALE == RESCALE - 1 and i + 3 < nsteps:
                    rescale(0, a, i)
                    rescale(1, uv, i)
            a = apply_pending(0, a, "alpha")
            uv = apply_pending(1, uv, "uv")
            # final v_{TM} = E uv_{TM+1} ; Z = <a, v_TM>
            pv = cp.tile([K, BL], F32, tag="pv")
            nc.tensor.matmul(pv[:], eexpT[:], uv[:], start=True, stop=True)
            q = work.tile([K, BL], F32, tag="q")
            nc.vector.tensor_tensor(q[:], a[:], pv[:], OP.mult)
            psz = sp.tile([1, BL], F32, tag="psz")
            nc.tensor.matmul(psz[:], ones20[:], q[:], start=True, stop=True)
            lnz = work.tile([1, BL], F32, tag="lnz")
            nc.scalar.activation(lnz[:], psz[:], AF.Ln)
            logz = work.tile([1, BL], F32, tag="logz")
            nc.vector.tensor_tensor(logz[:], lnz[:], laccs[0][:], OP.add)
            logz2 = work.tile([1, BL], F32, tag="logz2")
            nc.vector.tensor_tensor(logz2[:], logz[:], laccs[1][:], OP.add)
            nc.sync.dma_start(out=outm[1:2, :], in_=logz2[:])

            # ---- numerator finish ----
            # reduce nacc [K, NCH*BL] (chunk-major) -> [K, BL]
            cur = nacc
            width = NCH * BL
            while width > BL:
                half = width // 2
                red = work.tile([K, half], F32, tag="red")
                nc.vector.tensor_tensor(red[:], cur[:, 0:half],
                                        cur[:, half:width], OP.add)
                cur = red
                width = half
            qe = work.tile([K, BL], F32, tag="qe")
            nc.vector.tensor_tensor(
                qe[:], OHt[:, (nt - 1) * BL:nt * BL],
                end_sb[:, 0:1].to_broadcast([K, BL]), OP.mult)
            stot = work.tile([K, BL], F32, tag="stot")
            nc.vector.tensor_tensor(stot[:], cur[:, 0:BL], qe[:], OP.add)
            pss = sp.tile([1, BL], F32, tag="pss")
            nc.tensor.matmul(pss[:], ones20[:], stot[:], start=True, stop=True)
            score = work.tile([1, BL], F32, tag="score")
            nc.vector.tensor_copy(score[:], pss[:])
            nc.sync.dma_start(out=outm[0:1, :], in_=score[:])

    nc.compile()
    return nc


# ---------------------------------------------------------------------------
# Host side
# ---------------------------------------------------------------------------
_CACHE = {}


def _get_nc(nt):
    if nt not in _CACHE:
        _CACHE[nt] = build(nt)
    return _CACHE[nt]


def prep_inputs(sentences, tags, embed, Wih0, Whh0, b0, Wih1, Whh1, b1,
                Wout, bout, trans, start, end, nt=T):
    """Host-side marshalling: gate-split transposed bf16 weights."""
    f32 = np.float32
    bf16 = ml_dtypes.bfloat16
    gs = np.array([1.0, 1.0, 2.0, 1.0], f32)  # g-gate rows x2 (tanh trick)

    def wT(w, extra):
        # w [4H, D] -> [4, D, H], rows scaled by gs*extra, transposed
        out = np.empty((4, w.shape[1], H), f32)
        for g in range(4):
            out[g] = (w[g * H:(g + 1) * H] * (gs[g] * extra)).T
        return out

    wih0 = np.stack([wT(Wih0[d], 1.0) for d in range(2)]).astype(bf16)
    whh0 = np.stack([wT(Whh0[d], 2.0) for d in range(2)]).astype(bf16)
    b0v = np.stack([(b0[d].reshape(4, H) * gs[:, None]).reshape(4, 1, H)
                    for d in range(2)]).astype(bf16)
    wih1f = np.stack([wT(Wih1[d], 2.0) for d in range(2)])  # [2,4,2H,H]
    wih1 = np.ascontiguousarray(
        wih1f.reshape(2, 4, 2, H, H).transpose(0, 2, 1, 3, 4)).astype(bf16)
    whh1 = np.stack([wT(Whh1[d], 2.0) for d in range(2)]).astype(bf16)
    b1v = np.stack([(b1[d].reshape(4, H) * gs[:, None]).reshape(4, 1, H)
                    for d in range(2)]).astype(bf16)
    woutT = np.stack([np.ascontiguousarray((2.0 * Wout[:, :H]).T),
                      np.ascontiguousarray((2.0 * Wout[:, H:]).T)]).astype(bf16)
    shared = dict(
        embedb=np.ascontiguousarray(embed.astype(bf16)),
        wih0m=wih0, whh0m=whh0, b0m=b0v, wih1m=wih1, whh1m=whh1, b1m=b1v,
        woutm=woutT, boutv=bout.reshape(K, 1).astype(f32),
        transm=trans.astype(f32),
        transTm=np.ascontiguousarray(trans.T).astype(f32),
        transbm=trans.astype(bf16),
        startv=start.reshape(K, 1).astype(f32),
        endv=end.reshape(K, 1).astype(f32),
    )
    in_maps = []
    for c in range(NCORES):
        bsl = slice(c * BL, (c + 1) * BL)
        m = dict(shared)
        m["toks16"] = np.ascontiguousarray(
            sentences[bsl, :nt].astype(np.int16))
        m["tagsf"] = np.ascontiguousarray(
            tags[bsl, :nt].T.astype(f32).reshape(1, BL * nt))  # t-major
        in_maps.append(m)
    return in_maps


def run(inputs_np, nt=T, trace=False):
    nc = _get_nc(nt)
    in_maps = prep_inputs(
        inputs_np["sentences"], inputs_np["tags"], inputs_np["embed"],
        inputs_np["Wih0"], inputs_np["Whh0"], inputs_np["b0"],
        inputs_np["Wih1"], inputs_np["Whh1"], inputs_np["b1"],
        inputs_np["Wout"], inputs_np["bout"], inputs_np["trans"],
        inputs_np["start"], inputs_np["end"], nt=nt)
    res = run_bass_kernel_spmd(nc, in_maps, core_ids=list(range(NCORES)),
                               trace=trace)
    score = np.concatenate([res.results[c]["outm"][0] for c in range(NCORES)])
    logz = np.concatenate([res.results[c]["outm"][1] for c in range(NCORES)])
    loss = -np.mean(score - logz)
    return np.float32(loss), res


def kernel(**inputs):
    inputs_np = {k: np.asarray(v) for k, v in inputs.items()}
    loss, _ = run(inputs_np, nt=T)
    return np.asarray(loss, dtype=np.float32)


# revision 4
# speedup vs baseline: 1.8938x; 1.0022x over previous
"""BiLSTM-CRF loss kernel v2 for 8x Trainium2 NeuronCores (Bass/Tile).

Data-parallel over batch (16 sentences/core), feature-major layout:
hidden dim H=128 on partitions, batch in the free dim.

Per LSTM tick per direction the serial chain is only:
  4 tiny rec matmuls (bf16, 16 cols) -> sigmoid [128,64] (all 4 gates)
  -> u/fc/c~ (DVE+Pool) -> sigmoid(4c~) [128,16] -> h~ stt -> next matmul.
Input projections x@Wih + b are pre-accumulated into the same PSUM banks
8 ticks at a time by wide matmuls, so no per-tick add is needed.

Math identical to baseline: tanh via sigmoid (g-rows x2 in weights),
h~ = h/2 (x2 folded into Whh/Wih1/Wout), c~ = c/2, sc = sigmoid(4*c~).

CRF: exp-space with periodic rescaling, split into TWO concurrent chains
meeting in the middle:  Z = alpha_{M} . v_{M},  alpha forward from t=0,
v backward from t=T-1 (v_t = E @ (v_{t+1} * expem_{t+1})).

Numerator: one-hot tags OH [K,NTB]; G = Wout.h1+bout+trans^T@OH(shift)
accumulated in PSUM; score_b = sum_t <G(:,t,b), OH(:,t,b)> + end-term.
Emissions/numerator work drips into the layer-1 scan as chunks of h1
become ready.
"""

import sys

sys.path.insert(0, "/opt/trn_rl_repo")

import contextlib

import numpy as np
import ml_dtypes

import concourse.bass as bass
import concourse.tile as tile
from concourse import bacc, mybir
from concourse.bass_utils import run_bass_kernel_spmd

F32 = mybir.dt.float32
BF16 = mybir.dt.bfloat16
I16 = mybir.dt.int16
AF = mybir.ActivationFunctionType
OP = mybir.AluOpType

NCORES = 8
B, T, E, H, K, V = 128, 512, 128, 128, 20, 30000
BL = B // NCORES        # 16 sentences per core
GRP = 8                 # ticks per PSUM z-bank
CT = 32                 # ticks per emissions chunk (512 cols)
RESCALE = 8


def build(nt=T):
    nc = bacc.Bacc("TRN2", target_bir_lowering=False, debug=False,
                   num_devices=NCORES)
    NTB = nt * BL
    NG = nt // GRP
    NCH = NTB // (CT * BL)          # emissions chunks

    # ---- DRAM I/O ----
    embedb = nc.dram_tensor("embedb", [V, E], BF16, kind="ExternalInput")
    toks16 = nc.dram_tensor("toks16", [BL, nt], I16, kind="ExternalInput")
    tagsf = nc.dram_tensor("tagsf", [1, NTB], F32, kind="ExternalInput")  # t-major
    wih0m = nc.dram_tensor("wih0m", [2, 4, E, H], BF16, kind="ExternalInput")
    whh0m = nc.dram_tensor("whh0m", [2, 4, H, H], BF16, kind="ExternalInput")
    b0m = nc.dram_tensor("b0m", [2, 4, 1, H], BF16, kind="ExternalInput")
    wih1m = nc.dram_tensor("wih1m", [2, 2, 4, H, H], BF16, kind="ExternalInput")
    whh1m = nc.dram_tensor("whh1m", [2, 4, H, H], BF16, kind="ExternalInput")
    b1m = nc.dram_tensor("b1m", [2, 4, 1, H], BF16, kind="ExternalInput")
    woutm = nc.dram_tensor("woutm", [2, H, K], BF16, kind="ExternalInput")
    boutv = nc.dram_tensor("boutv", [K, 1], F32, kind="ExternalInput")
    transm = nc.dram_tensor("transm", [K, K], F32, kind="ExternalInput")
    transTm = nc.dram_tensor("transTm", [K, K], F32, kind="ExternalInput")
    transbm = nc.dram_tensor("transbm", [K, K], BF16, kind="ExternalInput")
    startv = nc.dram_tensor("startv", [K, 1], F32, kind="ExternalInput")
    endv = nc.dram_tensor("endv", [K, 1], F32, kind="ExternalInput")
    outm = nc.dram_tensor("outm", [2, BL], F32, kind="ExternalOutput")

    with tile.TileContext(nc) as tc, contextlib.ExitStack() as ctx:
        big = ctx.enter_context(tc.tile_pool(name="big", bufs=1))
        wp = ctx.enter_context(tc.tile_pool(name="wp", bufs=1))
        work = ctx.enter_context(tc.tile_pool(name="work", bufs=4))
        gw = ctx.enter_context(tc.tile_pool(name="gw", bufs=2))
        stp = ctx.enter_context(tc.tile_pool(name="stp", bufs=3))

        # ---------------- P0: constants, weights, gather ----------------
        idx = wp.tile([128, nt], I16, tag="idx")
        nc.gpsimd.memset(idx[:], 0)
        nc.sync.dma_start(out=idx[0:BL, :], in_=toks16[:, :])

        def load_w(name, dram_ap, shape, dt=BF16):
            t = wp.tile(shape, dt, tag=name)
            nc.sync.dma_start(out=t[:], in_=dram_ap)
            return t

        # L0 weights first (single wide DMAs — each dma_start holds the SP
        # sequencer for 650ns, so per-gate loads would serialize ~16us).
        def load_wide(name, dram4, shape2, perm):
            t = wp.tile(shape2, BF16, tag=name)
            t4 = t.rearrange("e (d g h) -> e d g h", d=2, g=4)
            nc.sync.dma_start(out=t4, in_=dram4.rearrange(perm))
            return t
        wih0t = load_wide("wih0t", wih0m, [E, 8 * H], "d g e h -> e d g h")
        whh0t = load_wide("whh0t", whh0m, [H, 8 * H], "d g e h -> e d g h")
        b0t = load_wide("b0t", b0m, [1, 8 * H], "d g o h -> o d g h")
        wih0 = [[wih0t[:, (d * 4 + g) * H:(d * 4 + g + 1) * H]
                 for g in range(4)] for d in range(2)]
        whh0 = [[whh0t[:, (d * 4 + g) * H:(d * 4 + g + 1) * H]
                 for g in range(4)] for d in range(2)]
        b0 = [[b0t[:, (d * 4 + g) * H:(d * 4 + g + 1) * H]
               for g in range(4)] for d in range(2)]

        onesb = wp.tile([1, GRP * BL], BF16, tag="onesb")
        nc.vector.memset(onesb[:], 1.0)
        ones20 = wp.tile([K, 1], F32, tag="ones20")
        nc.vector.memset(ones20[:], 1.0)
        ones2020 = wp.tile([K, K], F32, tag="ones2020")
        nc.vector.memset(ones2020[:], 1.0)
        zeros16 = wp.tile([128, BL], BF16, tag="zeros16")
        nc.vector.memset(zeros16[:], 0.0)
        iota20 = wp.tile([K, 1], mybir.dt.int32, tag="iota20i")
        nc.gpsimd.iota(iota20[:], pattern=[[0, 1]], base=0,
                       channel_multiplier=1)
        iota20f = wp.tile([K, 1], F32, tag="iota20f")
        nc.vector.tensor_copy(iota20f[:], iota20[:])
        # Embedding gather: xT [E=128, NTB] bf16, col = t*BL + b (t-major)
        xT = big.tile([128, NTB], BF16, tag="xT")
        xT3 = xT.rearrange("p (o c) -> p o c", o=1)
        GCH = 256
        ngch = max(1, NTB // GCH)
        # interleave from both ends so fwd (head) and bwd (tail) scans can
        # start before the full gather completes
        order = []
        for i in range((ngch + 1) // 2):
            order.append(ngch - 1 - i)
            if i != ngch - 1 - i:
                order.append(i)
        for g in order:
            cw = min(GCH, NTB)
            nc.gpsimd.dma_gather(
                xT3[:, :, g * cw:(g + 1) * cw], embedb[:, :],
                idx[:, g * (cw // BL):(g + 1) * (cw // BL)],
                cw, cw, E, transpose=True)

        # deferred weight loads (L1 / emissions / CRF), batched likewise
        wih1t = wp.tile([H, 16 * H], BF16, tag="wih1t")
        nc.sync.dma_start(
            out=wih1t.rearrange("e (d s g h) -> e d s g h", d=2, s=2, g=4),
            in_=wih1m.rearrange("d s g e h -> e d s g h"))
        whh1t = load_wide("whh1t", whh1m, [H, 8 * H], "d g e h -> e d g h")
        b1t = load_wide("b1t", b1m, [1, 8 * H], "d g o h -> o d g h")
        wih1 = [[[wih1t[:, ((d * 2 + s) * 4 + g) * H:
                        ((d * 2 + s) * 4 + g + 1) * H]
                  for g in range(4)] for s in range(2)] for d in range(2)]
        whh1 = [[whh1t[:, (d * 4 + g) * H:(d * 4 + g + 1) * H]
                 for g in range(4)] for d in range(2)]
        b1 = [[b1t[:, (d * 4 + g) * H:(d * 4 + g + 1) * H]
               for g in range(4)] for d in range(2)]
        wout_sb = [load_w(f"wout_{d}", woutm[d], [H, K]) for d in range(2)]
        bout_sb = load_w("bout", boutv[:, :], [K, 1], F32)
        trans_sb = load_w("trans", transm[:, :], [K, K], F32)
        transT_sb = load_w("transT", transTm[:, :], [K, K], F32)
        transb_sb = load_w("transb", transbm[:, :], [K, K], BF16)
        start_sb = load_w("start", startv[:, :], [K, 1], F32)
        end_sb = load_w("end", endv[:, :], [K, 1], F32)
        eexp = wp.tile([K, K], F32, tag="eexp")
        nc.scalar.activation(eexp[:], trans_sb[:], AF.Exp)
        eexpT = wp.tile([K, K], F32, tag="eexpT")
        nc.scalar.activation(eexpT[:], transT_sb[:], AF.Exp)
        expstart = wp.tile([K, 1], F32, tag="expstart")
        nc.scalar.activation(expstart[:], start_sb[:], AF.Exp)
        expend = wp.tile([K, 1], F32, tag="expend")
        nc.scalar.activation(expend[:], end_sb[:], AF.Exp)

        # tags -> one-hot OH [K, NTB] bf16 (built in chunks, dripped)
        tags_rep = big.tile([K, NTB], F32, tag="expem", name="tags_rep")
        nc.sync.dma_start(out=tags_rep[:],
                          in_=tagsf[0:1, :].to_broadcast([K, NTB]))
        OHt = big.tile([K, NTB], BF16, tag="OH")

        drip = []   # queue of zero-arg emit callbacks, popped ~2/step

        CHW = CT * BL  # 512 cols per chunk
        for c in range(NCH):
            def mk_oh(c=c):
                sl = slice(c * CHW, (c + 1) * CHW)
                nc.vector.tensor_tensor(
                    OHt[:, sl], iota20f[:, 0:1].to_broadcast([K, CHW]),
                    tags_rep[:, sl], OP.is_equal)
            drip.append(mk_oh)

        # Histories (bf16, feature-major, t-major cols)
        h0T = [big.tile([H, NTB], BF16, tag=f"h0T{d}", name=f"h0T{d}")
               for d in range(2)]
        h1T = [big.tile([H, NTB], BF16, tag=f"h1T{d}", name=f"h1T{d}")
               for d in range(2)]
        expem = big.tile([K, NTB], F32, tag="expem", name="expem")
        emfull = big.tile([K, NTB], F32, tag="em")
        nacc = wp.tile([K, max(1, NCH) * BL], F32, tag="nacc")

        # ---------------- the scan ----------------
        def scan_layer(layer, srcs, WIH, WHH, BB, hist_out, extra_cb=None):
            """srcs: list of full-width [128, NTB] moving tiles (1 or 2).
            WIH[d][s][g], WHH[d][g], BB[d][g]."""
            with tc.tile_pool(name=f"zp{layer}0", bufs=2, space="PSUM") as zf, \
                 tc.tile_pool(name=f"zp{layer}1", bufs=2, space="PSUM") as zb:
                zp = [zf, zb]
                banks = [None, None]
                cprev = []
                for d in range(2):
                    cp0 = stp.tile([H, BL], F32, tag=f"c{layer}{d}")
                    nc.vector.memset(cp0[:], 0.0)
                    cprev.append(cp0)

                def precompute(d, k):
                    bank = zp[d].tile([128, GRP * 64], F32, tag=f"z{d}")
                    t0 = k * GRP if d == 0 else nt - (k + 1) * GRP
                    cols = slice(t0 * BL, (t0 + GRP) * BL)
                    b3 = bank.rearrange("p (t c) -> p t c", c=64)
                    for si in range(len(srcs)):
                        for g in range(4):
                            nc.tensor.matmul(
                                b3[:, :, g * BL:(g + 1) * BL],
                                WIH[d][si][g][:], srcs[si][:, cols],
                                start=(si == 0), stop=False,
                                skip_group_check=True)
                    for g in range(4):
                        nc.tensor.matmul(
                            b3[:, :, g * BL:(g + 1) * BL],
                            BB[d][g][:], onesb[:, :],
                            start=False, stop=False, skip_group_check=True)
                    return bank

                banks[0] = precompute(0, 0)
                banks[1] = precompute(1, 0)
                nbanks = [None, None]
                for n in range(nt):
                    if n % GRP == 0 and n + GRP < nt:
                        nbanks[0] = precompute(0, n // GRP + 1)
                        nbanks[1] = precompute(1, n // GRP + 1)
                    tt_ = [n, nt - 1 - n]
                    toffs = [n % GRP, GRP - 1 - (n % GRP)]
                    # 1) recurrent matmuls (both dirs)
                    import os as _os
                    for d in range(2):
                        t_ = tt_[d]
                        bank = banks[d]
                        if n == 0 or _os.environ.get("KK_BREAK_H"):
                            hprev = zeros16[:, :]
                        else:
                            tp = t_ + (-1 if d == 0 else 1)
                            hprev = hist_out[d][:, tp * BL:(tp + 1) * BL]
                        for g in range(4):
                            o = toffs[d] * 64 + g * BL
                            nc.tensor.matmul(
                                bank[:, o:o + BL], WHH[d][g][:], hprev,
                                start=False, stop=True,
                                skip_group_check=True)
                    # 2) gate sigmoid (all 4 gates, one instr per dir)
                    ss = []
                    for d in range(2):
                        s = work.tile([H, 64], F32, tag=f"s{d}")
                        o = toffs[d] * 64
                        nc.scalar.activation(s[:], banks[d][:, o:o + 64],
                                             AF.Sigmoid)
                        ss.append(s)
                    # 3-5) per-chain grouped: u, fc, c~
                    VAR = _os.environ.get("KK_CVAR", "B")
                    us, fcs = [], []
                    for d in range(2):
                        u = work.tile([H, BL], F32, tag=f"u{d}")
                        nc.vector.scalar_tensor_tensor(
                            u[:], ss[d][:, 2 * BL:3 * BL], -0.5,
                            ss[d][:, 0:BL], OP.add, OP.mult)
                        us.append(u)
                        fc = work.tile([H, BL], F32, tag=f"fc{d}")
                        fc_eng = nc.vector if VAR == "B" else nc.gpsimd
                        fc_eng.tensor_tensor(
                            fc[:], ss[d][:, BL:2 * BL], cprev[d][:], OP.mult)
                        fcs.append(fc)
                        cn = stp.tile([H, BL], F32, tag=f"c{layer}{d}")
                        u2 = fc if _os.environ.get("KK_SKIP_U") else u
                        c_eng = nc.gpsimd if VAR == "C" else nc.vector
                        c_eng.tensor_tensor(cn[:], fc[:], u2[:], OP.add)
                        cprev[d] = cn
                    # 6) sc = sigmoid(4 c~)
                    scs = []
                    for d in range(2):
                        sc = work.tile([H, BL], F32, tag=f"sc{d}")
                        src = cp0 if _os.environ.get("KK_SKIP_SC") \
                            else cprev[d]
                        nc.scalar.activation(sc[:], src[:], AF.Sigmoid,
                                             scale=4.0)
                        scs.append(sc)
                    # 7) h~ = (sc - .5) * so  -> bf16 hist (read by next mm)
                    for d in range(2):
                        t_ = tt_[d]
                        nc.vector.scalar_tensor_tensor(
                            hist_out[d][:, t_ * BL:(t_ + 1) * BL],
                            scs[d][:], -0.5, ss[d][:, 3 * BL:4 * BL],
                            OP.add, OP.mult)
                    if n % GRP == GRP - 1:
                        banks[0], banks[1] = nbanks[0], nbanks[1]
                    if extra_cb is not None:
                        extra_cb(n)
                    for _ in range(2):
                        if drip:
                            drip.pop(0)()

        scan_layer(0, [xT], [[wih0[d]] for d in range(2)], whh0, b0, h0T)

        # ------- emissions + numerator chunks, dripped into L1 -------
        em_ctx = contextlib.ExitStack()
        ep = em_ctx.enter_context(tc.tile_pool(name="ep", bufs=2, space="PSUM"))
        sp1 = em_ctx.enter_context(tc.tile_pool(name="sp1", bufs=2,
                                                space="PSUM"))

        def emit_chunk_ops(c):
            """Ops for emissions chunk c (cols [c*CHW, (c+1)*CHW))."""
            sl = slice(c * CHW, (c + 1) * CHW)
            ops = []
            pe_box = {}

            def op_mm():
                pe = ep.tile([K, CHW], F32, tag="pe")
                nc.tensor.matmul(pe[:], wout_sb[0][:], h1T[0][:, sl],
                                 start=True, stop=False)
                nc.tensor.matmul(pe[:], wout_sb[1][:], h1T[1][:, sl],
                                 start=False, stop=True)
                pe_box["pe"] = pe
            ops.append(op_mm)

            def op_g0():
                # em (+bout) -> persistent SBUF; Identity shares the Sigmoid
                # act table so no table reload mid-scan. Exp deferred to one
                # big post-scan instruction.
                nc.scalar.activation(emfull[:, sl], pe_box["pe"][:],
                                     AF.Identity, bias=bout_sb[:, 0:1])
            ops.append(op_g0)

            def op_s1():
                # S1[j, col] = trans[tag_{col-BL}, j] for col >= BL
                lo = c * CHW
                s1 = sp1.tile([K, CHW], F32, tag="s1")
                if c == 0:
                    nc.tensor.matmul(s1[:, BL:], transb_sb[:],
                                     OHt[:, 0:CHW - BL], start=True, stop=True)
                else:
                    nc.tensor.matmul(s1[:], transb_sb[:],
                                     OHt[:, lo - BL:lo + CHW - BL],
                                     start=True, stop=True)
                pe_box["s1"] = s1
            ops.append(op_s1)

            HW2 = CHW // 2

            def op_gf1():
                gf = gw.tile([K, CHW], F32, tag="gf")
                g0, s1 = emfull[:, sl], pe_box["s1"]
                if c == 0:
                    # t=0: G = em + start (no trans term)
                    nc.gpsimd.tensor_tensor(
                        gf[:, 0:BL], g0[:, 0:BL],
                        start_sb[:, 0:1].to_broadcast([K, BL]), OP.add)
                    nc.vector.tensor_tensor(gf[:, BL:HW2], g0[:, BL:HW2],
                                            s1[:, BL:HW2], OP.add)
                else:
                    nc.vector.tensor_tensor(gf[:, 0:HW2], g0[:, 0:HW2],
                                            s1[:, 0:HW2], OP.add)
                pe_box["gf"] = gf
            ops.append(op_gf1)

            def op_gf2():
                gf, s1 = pe_box["gf"], pe_box["s1"]
                g0 = emfull[:, sl]
                nc.vector.tensor_tensor(gf[:, HW2:], g0[:, HW2:],
                                        s1[:, HW2:], OP.add)
            ops.append(op_gf2)

            for b in range(BL):
                def op_nb(b=b):
                    gf = pe_box["gf"]
                    g3 = gf.rearrange("p (t b) -> p t b", b=BL)
                    o3 = OHt[:, sl].rearrange("p (t b) -> p t b", b=BL)
                    dump = gw.tile([K, CT], F32, tag="dump")
                    d3 = dump.rearrange("p (t o) -> p t o", o=1)
                    nc.vector.scalar_tensor_tensor(
                        d3[:], g3[:, :, b:b + 1], 0.0, o3[:, :, b:b + 1],
                        OP.add, OP.mult,
                        accum_out=nacc[:, c * BL + b:c * BL + b + 1])
                ops.append(op_nb)
            return ops

        # chunk readiness: step max(CT*(c+1)-1, nt-1-CT*c)
        import os as _os
        pend = {}
        for c in range(NCH):
            if _os.environ.get("KK_EMIT_LATE"):
                rc = nt - 1
            else:
                rc = max(CT * (c + 1) - 1, nt - 1 - CT * c)
            pend.setdefault(rc, []).extend(emit_chunk_ops(c))

        def l1_cb(n):
            if n in pend:
                drip.extend(pend.pop(n))

        scan_layer(1, [h0T[0], h0T[1]], wih1, whh1, b1, h1T, extra_cb=l1_cb)
        for f in drip:
            f()
        drip.clear()
        em_ctx.close()
        # one Exp for the whole emissions tensor (single act-table switch)
        nc.scalar.activation(expem[:], emfull[:], AF.Exp)

        # ---------------- CRF: two concurrent chains ----------------
        TM = nt // 2 - 1   # alpha meets v at t=TM
        with tc.tile_pool(name="cp", bufs=2, space="PSUM") as cp, \
             tc.tile_pool(name="sp", bufs=1, space="PSUM") as sp:
            a = stp.tile([K, BL], F32, tag="alpha")
            nc.vector.tensor_tensor(
                a[:], expem[:, 0:BL],
                expstart[:, 0:1].to_broadcast([K, BL]), OP.mult)
            uv = stp.tile([K, BL], F32, tag="uv")
            nc.vector.tensor_tensor(
                uv[:], expem[:, (nt - 1) * BL:nt * BL],
                expend[:, 0:1].to_broadcast([K, BL]), OP.mult)
            la = stp.tile([1, BL], F32, tag="lacc0")
            nc.vector.memset(la[:], 0.0)
            lv = stp.tile([1, BL], F32, tag="lacc1")
            nc.vector.memset(lv[:], 0.0)
            laccs = [la, lv]
            pendings = [None, None]

            def rescale(hh, cur, i):
                ps = sp.tile([K, BL], F32, tag=f"ps{hh}")
                nc.tensor.matmul(ps[:], ones2020[:], cur[:],
                                 start=True, stop=True)
                sinv = work.tile([K, BL], F32, tag=f"sinv{hh}")
                nc.vector.reciprocal(sinv[:], ps[:])
                lt = work.tile([1, BL], F32, tag=f"lt{hh}")
                nc.scalar.activation(lt[:], ps[0:1, :], AF.Ln)
                lnew = stp.tile([1, BL], F32, tag=f"lacc{hh}")
                nc.vector.tensor_tensor(lnew[:], laccs[hh][:], lt[:], OP.add)
                laccs[hh] = lnew
                pendings[hh] = (sinv, i + 2)

            def apply_pending(hh, cur, tag):
                if pendings[hh] is not None:
                    nw = stp.tile([K, BL], F32, tag=tag)
                    nc.vector.tensor_tensor(nw[:], cur[:],
                                            pendings[hh][0][:], OP.mult)
                    pendings[hh] = None
                    return nw
                return cur

            nsteps = TM   # alpha steps t=1..TM ; v steps t=nt-2..TM+1
            for i in range(nsteps):
                ta = 1 + i
                tv = nt - 2 - i
                # alpha: pa = Eexp^T a ; a = pa * expem_ta
                pa = cp.tile([K, BL], F32, tag="pa")
                nc.tensor.matmul(pa[:], eexp[:], a[:], start=True, stop=True)
                # v: pv = E uv ; uv = pv * expem_tv
                pv = cp.tile([K, BL], F32, tag="pv")
                nc.tensor.matmul(pv[:], eexpT[:], uv[:], start=True, stop=True)
                an = stp.tile([K, BL], F32, tag="alpha")
                nc.vector.tensor_tensor(an[:], pa[:],
                                        expem[:, ta * BL:(ta + 1) * BL],
                                        OP.mult)
                a = an
                un = stp.tile([K, BL], F32, tag="uv")
                nc.vector.tensor_tensor(un[:], pv[:],
                                        expem[:, tv * BL:(tv + 1) * BL],
                                        OP.mult)
                uv = un
                if pendings[0] is not None and i >= pendings[0][1]:
                    a = apply_pending(0, a, "alpha")
                if pendings[1] is not None and i >= pendings[1][1]:
                    uv = apply_pending(1, uv, "uv")
                if i % RESCALE == RESCALE - 1 and i + 3 < nsteps:
                    rescale(0, a, i)
                    rescale(1, uv, i)
            a = apply_pending(0, a, "alpha")
            uv = apply_pending(1, uv, "uv")
            # final v_{TM} = E uv_{TM+1} ; Z = <a, v_TM>
            pv = cp.tile([K, BL], F32, tag="pv")
            nc.tensor.matmul(pv[:], eexpT[:], uv[:], start=True, stop=True)
            q = work.tile([K, BL], F32, tag="q")
            nc.vector.tensor_tensor(q[:], a[:], pv[:], OP.mult)
            psz = sp.tile([1, BL], F32, tag="psz")
            nc.tensor.matmul(psz[:], ones20[:], q[:], start=True, stop=True)
            lnz = work.tile([1, BL], F32, tag="lnz")
            nc.scalar.activation(lnz[:], psz[:], AF.Ln)
            logz = work.tile([1, BL], F32, tag="logz")
            nc.vector.tensor_tensor(logz[:], lnz[:], laccs[0][:], OP.add)
            logz2 = work.tile([1, BL], F32, tag="logz2")
            nc.vector.tensor_tensor(logz2[:], logz[:], laccs[1][:], OP.add)
            nc.sync.dma_start(out=outm[1:2, :], in_=logz2[:])

            # ---- numerator finish ----
            # reduce nacc [K, NCH*BL] (chunk-major) -> [K, BL]
            cur = nacc
            width = NCH * BL
            while width > BL:
                half = width // 2
                red = work.tile([K, half], F32, tag="red")
                nc.vector.tensor_tensor(red[:], cur[:, 0:half],
                                        cur[:, half:width], OP.add)
                cur = red
                width = half
            qe = work.tile([K, BL], F32, tag="qe")
            nc.vector.tensor_tensor(
                qe[:], OHt[:, (nt - 1) * BL:nt * BL],
                end_sb[:, 0:1].to_broadcast([K, BL]), OP.mult)
            stot = work.tile([K, BL], F32, tag="stot")
            nc.vector.tensor_tensor(stot[:], cur[:, 0:BL], qe[:], OP.add)
            pss = sp.tile([1, BL], F32, tag="pss")
            nc.tensor.matmul(pss[:], ones20[:], stot[:], start=True, stop=True)
            score = work.tile([1, BL], F32, tag="score")
            nc.vector.tensor_copy(score[:], pss[:])
            nc.sync.dma_start(out=outm[0:1, :], in_=score[:])

    nc.compile()
    return nc


# ---------------------------------------------------------------------------
# Host side
# ---------------------------------------------------------------------------
_CACHE = {}


def _get_nc(nt):
    if nt not in _CACHE:
        _CACHE[nt] = build(nt)
    return _CACHE[nt]


def prep_inputs(sentences, tags, embed, Wih0, Whh0, b0, Wih1, Whh1, b1,
                Wout, bout, trans, start, end, nt=T):
    """Host-side marshalling: gate-split transposed bf16 weights."""
    f32 = np.float32
    bf16 = ml_dtypes.bfloat16
    gs = np.array([1.0, 1.0, 2.0, 1.0], f32)  # g-gate rows x2 (tanh trick)

    def wT(w, extra):
        # w [4H, D] -> [4, D, H], rows scaled by gs*extra, transposed
        out = np.empty((4, w.shape[1], H), f32)
        for g in range(4):
            out[g] = (w[g * H:(g + 1) * H] * (gs[g] * extra)).T
        return out

    wih0 = np.stack([wT(Wih0[d], 1.0) for d in range(2)]).astype(bf16)
    whh0 = np.stack([wT(Whh0[d], 2.0) for d in range(2)]).astype(bf16)
    b0v = np.stack([(b0[d].reshape(4, H) * gs[:, None]).reshape(4, 1, H)
                    for d in range(2)]).astype(bf16)
    wih1f = np.stack([wT(Wih1[d], 2.0) for d in range(2)])  # [2,4,2H,H]
    wih1 = np.ascontiguousarray(
        wih1f.reshape(2, 4, 2, H, H).transpose(0, 2, 1, 3, 4)).astype(bf16)
    whh1 = np.stack([wT(Whh1[d], 2.0) for d in range(2)]).astype(bf16)
    b1v = np.stack([(b1[d].reshape(4, H) * gs[:, None]).reshape(4, 1, H)
                    for d in range(2)]).astype(bf16)
    woutT = np.stack([np.ascontiguousarray((2.0 * Wout[:, :H]).T),
                      np.ascontiguousarray((2.0 * Wout[:, H:]).T)]).astype(bf16)
    shared = dict(
        embedb=np.ascontiguousarray(embed.astype(bf16)),
        wih0m=wih0, whh0m=whh0, b0m=b0v, wih1m=wih1, whh1m=whh1, b1m=b1v,
        woutm=woutT, boutv=bout.reshape(K, 1).astype(f32),
        transm=trans.astype(f32),
        transTm=np.ascontiguousarray(trans.T).astype(f32),
        transbm=trans.astype(bf16),
        startv=start.reshape(K, 1).astype(f32),
        endv=end.reshape(K, 1).astype(f32),
    )
    in_maps = []
    for c in range(NCORES):
        bsl = slice(c * BL, (c + 1) * BL)
        m = dict(shared)
        m["toks16"] = np.ascontiguousarray(
            sentences[bsl, :nt].astype(np.int16))
        m["tagsf"] = np.ascontiguousarray(
            tags[bsl, :nt].T.astype(f32).reshape(1, BL * nt))  # t-major
        in_maps.append(m)
    return in_maps


def run(inputs_np, nt=T, trace=False):
    nc = _get_nc(nt)
    in_maps = prep_inputs(
        inputs_np["sentences"], inputs_np["tags"], inputs_np["embed"],
        inputs_np["Wih0"], inputs_np["Whh0"], inputs_np["b0"],
        inputs_np["Wih1"], inputs_np["Whh1"], inputs_np["b1"],
        inputs_np["Wout"], inputs_np["bout"], inputs_np["trans"],
        inputs_np["start"], inputs_np["end"], nt=nt)
    res = run_bass_kernel_spmd(nc, in_maps, core_ids=list(range(NCORES)),
                               trace=trace)
    score = np.concatenate([res.results[c]["outm"][0] for c in range(NCORES)])
    logz = np.concatenate([res.results[c]["outm"][1] for c in range(NCORES)])
    loss = -np.mean(score - logz)
    return np.float32(loss), res


def kernel(**inputs):
    inputs_np = {k: np.asarray(v) for k, v in inputs.items()}
    loss, _ = run(inputs_np, nt=T)
    return np.asarray(loss, dtype=np.float32)


# revision 5
# speedup vs baseline: 1.8995x; 1.0030x over previous
"""BiLSTM-CRF loss kernel v2 for 8x Trainium2 NeuronCores (Bass/Tile).

Data-parallel over batch (16 sentences/core), feature-major layout:
hidden dim H=128 on partitions, batch in the free dim.

Per LSTM tick per direction the serial chain is only:
  4 tiny rec matmuls (bf16, 16 cols) -> sigmoid [128,64] (all 4 gates)
  -> u/fc/c~ (DVE+Pool) -> sigmoid(4c~) [128,16] -> h~ stt -> next matmul.
Input projections x@Wih + b are pre-accumulated into the same PSUM banks
8 ticks at a time by wide matmuls, so no per-tick add is needed.

Math identical to baseline: tanh via sigmoid (g-rows x2 in weights),
h~ = h/2 (x2 folded into Whh/Wih1/Wout), c~ = c/2, sc = sigmoid(4*c~).

CRF: exp-space with periodic rescaling, split into TWO concurrent chains
meeting in the middle:  Z = alpha_{M} . v_{M},  alpha forward from t=0,
v backward from t=T-1 (v_t = E @ (v_{t+1} * expem_{t+1})).

Numerator: one-hot tags OH [K,NTB]; G = Wout.h1+bout+trans^T@OH(shift)
accumulated in PSUM; score_b = sum_t <G(:,t,b), OH(:,t,b)> + end-term.
Emissions/numerator work drips into the layer-1 scan as chunks of h1
become ready.
"""

import sys

sys.path.insert(0, "/opt/trn_rl_repo")

import contextlib

import numpy as np
import ml_dtypes

import concourse.bass as bass
import concourse.tile as tile
from concourse import bacc, mybir
from concourse.bass_utils import run_bass_kernel_spmd

F32 = mybir.dt.float32
BF16 = mybir.dt.bfloat16
I16 = mybir.dt.int16
AF = mybir.ActivationFunctionType
OP = mybir.AluOpType

NCORES = 8
B, T, E, H, K, V = 128, 512, 128, 128, 20, 30000
BL = B // NCORES        # 16 sentences per core
GRP = 8                 # ticks per PSUM z-bank
CT = 32                 # ticks per emissions chunk (512 cols)
RESCALE = 8


def build(nt=T):
    nc = bacc.Bacc("TRN2", target_bir_lowering=False, debug=False,
                   num_devices=NCORES)
    NTB = nt * BL
    NG = nt // GRP
    NCH = NTB // (CT * BL)          # emissions chunks

    # ---- DRAM I/O ----
    embedb = nc.dram_tensor("embedb", [V, E], BF16, kind="ExternalInput")
    toks16 = nc.dram_tensor("toks16", [BL, nt], I16, kind="ExternalInput")
    tagsf = nc.dram_tensor("tagsf", [1, NTB], F32, kind="ExternalInput")  # t-major
    wih0m = nc.dram_tensor("wih0m", [2, 4, E, H], BF16, kind="ExternalInput")
    whh0m = nc.dram_tensor("whh0m", [2, 4, H, H], BF16, kind="ExternalInput")
    b0m = nc.dram_tensor("b0m", [2, 4, 1, H], BF16, kind="ExternalInput")
    wih1m = nc.dram_tensor("wih1m", [2, 2, 4, H, H], BF16, kind="ExternalInput")
    whh1m = nc.dram_tensor("whh1m", [2, 4, H, H], BF16, kind="ExternalInput")
    b1m = nc.dram_tensor("b1m", [2, 4, 1, H], BF16, kind="ExternalInput")
    woutm = nc.dram_tensor("woutm", [2, H, K], BF16, kind="ExternalInput")
    boutv = nc.dram_tensor("boutv", [K, 1], F32, kind="ExternalInput")
    transm = nc.dram_tensor("transm", [K, K], F32, kind="ExternalInput")
    transTm = nc.dram_tensor("transTm", [K, K], F32, kind="ExternalInput")
    transbm = nc.dram_tensor("transbm", [K, K], BF16, kind="ExternalInput")
    startv = nc.dram_tensor("startv", [K, 1], F32, kind="ExternalInput")
    endv = nc.dram_tensor("endv", [K, 1], F32, kind="ExternalInput")
    outm = nc.dram_tensor("outm", [2, BL], F32, kind="ExternalOutput")

    with tile.TileContext(nc) as tc, contextlib.ExitStack() as ctx:
        big = ctx.enter_context(tc.tile_pool(name="big", bufs=1))
        wp = ctx.enter_context(tc.tile_pool(name="wp", bufs=1))
        work = ctx.enter_context(tc.tile_pool(name="work", bufs=4))
        gw = ctx.enter_context(tc.tile_pool(name="gw", bufs=2))
        stp = ctx.enter_context(tc.tile_pool(name="stp", bufs=3))

        # ---------------- P0: constants, weights, gather ----------------
        idx = wp.tile([128, nt], I16, tag="idx")
        nc.gpsimd.memset(idx[:], 0)
        nc.sync.dma_start(out=idx[0:BL, :], in_=toks16[:, :])

        def load_w(name, dram_ap, shape, dt=BF16):
            t = wp.tile(shape, dt, tag=name)
            nc.sync.dma_start(out=t[:], in_=dram_ap)
            return t

        # L0 weights first (single wide DMAs — each dma_start holds the SP
        # sequencer for 650ns, so per-gate loads would serialize ~16us).
        def load_wide(name, dram4, shape2, perm):
            t = wp.tile(shape2, BF16, tag=name)
            t4 = t.rearrange("e (d g h) -> e d g h", d=2, g=4)
            nc.sync.dma_start(out=t4, in_=dram4.rearrange(perm))
            return t
        wih0t = load_wide("wih0t", wih0m, [E, 8 * H], "d g e h -> e d g h")
        whh0t = load_wide("whh0t", whh0m, [H, 8 * H], "d g e h -> e d g h")
        b0t = load_wide("b0t", b0m, [1, 8 * H], "d g o h -> o d g h")
        wih0 = [[wih0t[:, (d * 4 + g) * H:(d * 4 + g + 1) * H]
                 for g in range(4)] for d in range(2)]
        whh0 = [[whh0t[:, (d * 4 + g) * H:(d * 4 + g + 1) * H]
                 for g in range(4)] for d in range(2)]
        b0 = [[b0t[:, (d * 4 + g) * H:(d * 4 + g + 1) * H]
               for g in range(4)] for d in range(2)]

        onesb = wp.tile([1, GRP * BL], BF16, tag="onesb")
        nc.vector.memset(onesb[:], 1.0)
        ones20 = wp.tile([K, 1], F32, tag="ones20")
        nc.vector.memset(ones20[:], 1.0)
        ones2020 = wp.tile([K, K], F32, tag="ones2020")
        nc.vector.memset(ones2020[:], 1.0)
        zeros16 = wp.tile([128, BL], BF16, tag="zeros16")
        nc.vector.memset(zeros16[:], 0.0)
        iota20 = wp.tile([K, 1], mybir.dt.int32, tag="iota20i")
        nc.gpsimd.iota(iota20[:], pattern=[[0, 1]], base=0,
                       channel_multiplier=1)
        iota20f = wp.tile([K, 1], F32, tag="iota20f")
        nc.vector.tensor_copy(iota20f[:], iota20[:])
        # Embedding gather: xT [E=128, NTB] bf16, col = t*BL + b (t-major)
        xT = big.tile([128, NTB], BF16, tag="xT")
        xT3 = xT.rearrange("p (o c) -> p o c", o=1)
        GCH = 256
        ngch = max(1, NTB // GCH)
        # interleave from both ends so fwd (head) and bwd (tail) scans can
        # start before the full gather completes
        order = []
        for i in range((ngch + 1) // 2):
            order.append(ngch - 1 - i)
            if i != ngch - 1 - i:
                order.append(i)
        for g in order:
            cw = min(GCH, NTB)
            nc.gpsimd.dma_gather(
                xT3[:, :, g * cw:(g + 1) * cw], embedb[:, :],
                idx[:, g * (cw // BL):(g + 1) * (cw // BL)],
                cw, cw, E, transpose=True)

        # deferred weight loads (L1 / emissions / CRF), batched likewise
        wih1t = wp.tile([H, 16 * H], BF16, tag="wih1t")
        nc.sync.dma_start(
            out=wih1t.rearrange("e (d s g h) -> e d s g h", d=2, s=2, g=4),
            in_=wih1m.rearrange("d s g e h -> e d s g h"))
        whh1t = load_wide("whh1t", whh1m, [H, 8 * H], "d g e h -> e d g h")
        b1t = load_wide("b1t", b1m, [1, 8 * H], "d g o h -> o d g h")
        wih1 = [[[wih1t[:, ((d * 2 + s) * 4 + g) * H:
                        ((d * 2 + s) * 4 + g + 1) * H]
                  for g in range(4)] for s in range(2)] for d in range(2)]
        whh1 = [[whh1t[:, (d * 4 + g) * H:(d * 4 + g + 1) * H]
                 for g in range(4)] for d in range(2)]
        b1 = [[b1t[:, (d * 4 + g) * H:(d * 4 + g + 1) * H]
               for g in range(4)] for d in range(2)]
        wout_sb = [load_w(f"wout_{d}", woutm[d], [H, K]) for d in range(2)]
        bout_sb = load_w("bout", boutv[:, :], [K, 1], F32)
        trans_sb = load_w("trans", transm[:, :], [K, K], F32)
        transT_sb = load_w("transT", transTm[:, :], [K, K], F32)
        transb_sb = load_w("transb", transbm[:, :], [K, K], BF16)
        start_sb = load_w("start", startv[:, :], [K, 1], F32)
        end_sb = load_w("end", endv[:, :], [K, 1], F32)
        eexp = wp.tile([K, K], F32, tag="eexp")
        nc.scalar.activation(eexp[:], trans_sb[:], AF.Exp)
        eexpT = wp.tile([K, K], F32, tag="eexpT")
        nc.scalar.activation(eexpT[:], transT_sb[:], AF.Exp)
        expstart = wp.tile([K, 1], F32, tag="expstart")
        nc.scalar.activation(expstart[:], start_sb[:], AF.Exp)
        expend = wp.tile([K, 1], F32, tag="expend")
        nc.scalar.activation(expend[:], end_sb[:], AF.Exp)

        # tags -> one-hot OH [K, NTB] bf16 (built in chunks, dripped)
        tags_rep = big.tile([K, NTB], F32, tag="expem", name="tags_rep")
        nc.sync.dma_start(out=tags_rep[:],
                          in_=tagsf[0:1, :].to_broadcast([K, NTB]))
        OHt = big.tile([K, NTB], BF16, tag="OH")

        drip = []   # queue of zero-arg emit callbacks, popped ~2/step

        CHW = CT * BL  # 512 cols per chunk
        for c in range(NCH):
            def mk_oh(c=c):
                sl = slice(c * CHW, (c + 1) * CHW)
                nc.vector.tensor_tensor(
                    OHt[:, sl], iota20f[:, 0:1].to_broadcast([K, CHW]),
                    tags_rep[:, sl], OP.is_equal)
            drip.append(mk_oh)

        # Histories (bf16, feature-major, t-major cols)
        h0T = [big.tile([H, NTB], BF16, tag=f"h0T{d}", name=f"h0T{d}")
               for d in range(2)]
        h1T = [big.tile([H, NTB], BF16, tag=f"h1T{d}", name=f"h1T{d}")
               for d in range(2)]
        expem = big.tile([K, NTB], F32, tag="expem", name="expem")
        emfull = big.tile([K, NTB], F32, tag="em")
        nacc = wp.tile([K, max(1, NCH) * BL], F32, tag="nacc")

        # ---------------- the scan ----------------
        def scan_layer(layer, srcs, WIH, WHH, BB, hist_out, extra_cb=None):
            """srcs: list of full-width [128, NTB] moving tiles (1 or 2).
            WIH[d][s][g], WHH[d][g], BB[d][g]."""
            with tc.tile_pool(name=f"zp{layer}0", bufs=2, space="PSUM") as zf, \
                 tc.tile_pool(name=f"zp{layer}1", bufs=2, space="PSUM") as zb:
                zp = [zf, zb]
                banks = [None, None]
                cprev = []
                for d in range(2):
                    cp0 = stp.tile([H, BL], F32, tag=f"c{layer}{d}")
                    nc.vector.memset(cp0[:], 0.0)
                    cprev.append(cp0)

                def precompute(d, k):
                    bank = zp[d].tile([128, GRP * 64], F32, tag=f"z{d}")
                    t0 = k * GRP if d == 0 else nt - (k + 1) * GRP
                    cols = slice(t0 * BL, (t0 + GRP) * BL)
                    b3 = bank.rearrange("p (t c) -> p t c", c=64)
                    for si in range(len(srcs)):
                        for g in range(4):
                            nc.tensor.matmul(
                                b3[:, :, g * BL:(g + 1) * BL],
                                WIH[d][si][g][:], srcs[si][:, cols],
                                start=(si == 0), stop=False,
                                skip_group_check=True)
                    for g in range(4):
                        nc.tensor.matmul(
                            b3[:, :, g * BL:(g + 1) * BL],
                            BB[d][g][:], onesb[:, :],
                            start=False, stop=False, skip_group_check=True)
                    return bank

                banks[0] = precompute(0, 0)
                banks[1] = precompute(1, 0)
                nbanks = [None, None]
                for n in range(nt):
                    if n % GRP == 0 and n + GRP < nt:
                        nbanks[0] = precompute(0, n // GRP + 1)
                        nbanks[1] = precompute(1, n // GRP + 1)
                    tt_ = [n, nt - 1 - n]
                    toffs = [n % GRP, GRP - 1 - (n % GRP)]
                    # 1) recurrent matmuls (both dirs)
                    import os as _os
                    for d in range(2):
                        t_ = tt_[d]
                        bank = banks[d]
                        if n == 0 or _os.environ.get("KK_BREAK_H"):
                            hprev = zeros16[:, :]
                        else:
                            tp = t_ + (-1 if d == 0 else 1)
                            hprev = hist_out[d][:, tp * BL:(tp + 1) * BL]
                        for g in range(4):
                            o = toffs[d] * 64 + g * BL
                            nc.tensor.matmul(
                                bank[:, o:o + BL], WHH[d][g][:], hprev,
                                start=False, stop=True,
                                skip_group_check=True)
                    # 2) gate sigmoid (all 4 gates, one instr per dir)
                    ss = []
                    for d in range(2):
                        s = work.tile([H, 64], F32, tag=f"s{d}")
                        o = toffs[d] * 64
                        nc.scalar.activation(s[:], banks[d][:, o:o + 64],
                                             AF.Sigmoid)
                        ss.append(s)
                    # 3-5) per-chain grouped: u, fc, c~
                    VAR = _os.environ.get("KK_CVAR", "B")
                    us, fcs = [], []
                    for d in range(2):
                        u = work.tile([H, BL], F32, tag=f"u{d}")
                        nc.vector.scalar_tensor_tensor(
                            u[:], ss[d][:, 2 * BL:3 * BL], -0.5,
                            ss[d][:, 0:BL], OP.add, OP.mult)
                        us.append(u)
                        fc = work.tile([H, BL], F32, tag=f"fc{d}")
                        fc_eng = nc.vector if VAR == "B" else nc.gpsimd
                        fc_eng.tensor_tensor(
                            fc[:], ss[d][:, BL:2 * BL], cprev[d][:], OP.mult)
                        fcs.append(fc)
                        cn = stp.tile([H, BL], F32, tag=f"c{layer}{d}")
                        u2 = fc if _os.environ.get("KK_SKIP_U") else u
                        c_eng = nc.gpsimd if VAR == "C" else nc.vector
                        c_eng.tensor_tensor(cn[:], fc[:], u2[:], OP.add)
                        cprev[d] = cn
                    # 6) sc = sigmoid(4 c~)
                    scs = []
                    for d in range(2):
                        sc = work.tile([H, BL], F32, tag=f"sc{d}")
                        src = cp0 if _os.environ.get("KK_SKIP_SC") \
                            else cprev[d]
                        nc.scalar.activation(sc[:], src[:], AF.Sigmoid,
                                             scale=4.0)
                        scs.append(sc)
                    # 7) h~ = (sc - .5) * so  -> bf16 hist (read by next mm)
                    for d in range(2):
                        t_ = tt_[d]
                        nc.vector.scalar_tensor_tensor(
                            hist_out[d][:, t_ * BL:(t_ + 1) * BL],
                            scs[d][:], -0.5, ss[d][:, 3 * BL:4 * BL],
                            OP.add, OP.mult)
                    if n % GRP == GRP - 1:
                        banks[0], banks[1] = nbanks[0], nbanks[1]
                    if extra_cb is not None:
                        extra_cb(n)
                    if drip:
                        drip.pop(0)()

        scan_layer(0, [xT], [[wih0[d]] for d in range(2)], whh0, b0, h0T)

        # ------- emissions + numerator chunks, dripped into L1 -------
        em_ctx = contextlib.ExitStack()
        ep = em_ctx.enter_context(tc.tile_pool(name="ep", bufs=2, space="PSUM"))
        sp1 = em_ctx.enter_context(tc.tile_pool(name="sp1", bufs=2,
                                                space="PSUM"))

        def emit_chunk_ops(c):
            """Ops for emissions chunk c (cols [c*CHW, (c+1)*CHW))."""
            sl = slice(c * CHW, (c + 1) * CHW)
            ops = []
            pe_box = {}

            def op_mm():
                pe = ep.tile([K, CHW], F32, tag="pe")
                nc.tensor.matmul(pe[:], wout_sb[0][:], h1T[0][:, sl],
                                 start=True, stop=False)
                nc.tensor.matmul(pe[:], wout_sb[1][:], h1T[1][:, sl],
                                 start=False, stop=True)
                pe_box["pe"] = pe
            ops.append(op_mm)

            def op_g0():
                # em (+bout) -> persistent SBUF; Identity shares the Sigmoid
                # act table so no table reload mid-scan. Exp deferred to one
                # big post-scan instruction.
                nc.scalar.activation(emfull[:, sl], pe_box["pe"][:],
                                     AF.Identity, bias=bout_sb[:, 0:1])
            ops.append(op_g0)

            def op_s1():
                # S1[j, col] = trans[tag_{col-BL}, j] for col >= BL
                lo = c * CHW
                s1 = sp1.tile([K, CHW], F32, tag="s1")
                if c == 0:
                    nc.tensor.matmul(s1[:, BL:], transb_sb[:],
                                     OHt[:, 0:CHW - BL], start=True, stop=True)
                else:
                    nc.tensor.matmul(s1[:], transb_sb[:],
                                     OHt[:, lo - BL:lo + CHW - BL],
                                     start=True, stop=True)
                pe_box["s1"] = s1
            ops.append(op_s1)

            HW2 = CHW // 2

            def op_gf1():
                gf = gw.tile([K, CHW], F32, tag="gf")
                g0, s1 = emfull[:, sl], pe_box["s1"]
                if c == 0:
                    # t=0: G = em + start (no trans term)
                    nc.gpsimd.tensor_tensor(
                        gf[:, 0:BL], g0[:, 0:BL],
                        start_sb[:, 0:1].to_broadcast([K, BL]), OP.add)
                    nc.vector.tensor_tensor(gf[:, BL:HW2], g0[:, BL:HW2],
                                            s1[:, BL:HW2], OP.add)
                else:
                    nc.vector.tensor_tensor(gf[:, 0:HW2], g0[:, 0:HW2],
                                            s1[:, 0:HW2], OP.add)
                pe_box["gf"] = gf
            ops.append(op_gf1)

            def op_gf2():
                gf, s1 = pe_box["gf"], pe_box["s1"]
                g0 = emfull[:, sl]
                nc.vector.tensor_tensor(gf[:, HW2:], g0[:, HW2:],
                                        s1[:, HW2:], OP.add)
            ops.append(op_gf2)

            for b in range(BL):
                def op_nb(b=b):
                    gf = pe_box["gf"]
                    g3 = gf.rearrange("p (t b) -> p t b", b=BL)
                    o3 = OHt[:, sl].rearrange("p (t b) -> p t b", b=BL)
                    dump = gw.tile([K, CT], F32, tag="dump")
                    d3 = dump.rearrange("p (t o) -> p t o", o=1)
                    nc.vector.scalar_tensor_tensor(
                        d3[:], g3[:, :, b:b + 1], 0.0, o3[:, :, b:b + 1],
                        OP.add, OP.mult,
                        accum_out=nacc[:, c * BL + b:c * BL + b + 1])
                ops.append(op_nb)
            return ops

        # chunk readiness: step max(CT*(c+1)-1, nt-1-CT*c)
        import os as _os
        pend = {}
        for c in range(NCH):
            if _os.environ.get("KK_EMIT_LATE"):
                rc = nt - 1
            else:
                rc = max(CT * (c + 1) - 1, nt - 1 - CT * c)
            pend.setdefault(rc, []).extend(emit_chunk_ops(c))

        def l1_cb(n):
            if n in pend:
                drip.extend(pend.pop(n))

        scan_layer(1, [h0T[0], h0T[1]], wih1, whh1, b1, h1T, extra_cb=l1_cb)
        for f in drip:
            f()
        drip.clear()
        em_ctx.close()
        # one Exp for the whole emissions tensor (single act-table switch)
        nc.scalar.activation(expem[:], emfull[:], AF.Exp)

        # ---------------- CRF: two concurrent chains ----------------
        TM = nt // 2 - 1   # alpha meets v at t=TM
        with tc.tile_pool(name="cp", bufs=2, space="PSUM") as cp, \
             tc.tile_pool(name="sp", bufs=1, space="PSUM") as sp:
            a = stp.tile([K, BL], F32, tag="alpha")
            nc.vector.tensor_tensor(
                a[:], expem[:, 0:BL],
                expstart[:, 0:1].to_broadcast([K, BL]), OP.mult)
            uv = stp.tile([K, BL], F32, tag="uv")
            nc.vector.tensor_tensor(
                uv[:], expem[:, (nt - 1) * BL:nt * BL],
                expend[:, 0:1].to_broadcast([K, BL]), OP.mult)
            la = stp.tile([1, BL], F32, tag="lacc0")
            nc.vector.memset(la[:], 0.0)
            lv = stp.tile([1, BL], F32, tag="lacc1")
            nc.vector.memset(lv[:], 0.0)
            laccs = [la, lv]
            pendings = [None, None]

            def rescale(hh, cur, i):
                ps = sp.tile([K, BL], F32, tag=f"ps{hh}")
                nc.tensor.matmul(ps[:], ones2020[:], cur[:],
                                 start=True, stop=True)
                sinv = work.tile([K, BL], F32, tag=f"sinv{hh}")
                nc.vector.reciprocal(sinv[:], ps[:])
                lt = work.tile([1, BL], F32, tag=f"lt{hh}")
                nc.scalar.activation(lt[:], ps[0:1, :], AF.Ln)
                lnew = stp.tile([1, BL], F32, tag=f"lacc{hh}")
                nc.vector.tensor_tensor(lnew[:], laccs[hh][:], lt[:], OP.add)
                laccs[hh] = lnew
                pendings[hh] = (sinv, i + 2)

            def apply_pending(hh, cur, tag):
                if pendings[hh] is not None:
                    nw = stp.tile([K, BL], F32, tag=tag)
                    nc.vector.tensor_tensor(nw[:], cur[:],
                                            pendings[hh][0][:], OP.mult)
                    pendings[hh] = None
                    return nw
                return cur

            nsteps = TM   # alpha steps t=1..TM ; v steps t=nt-2..TM+1
            for i in range(nsteps):
                ta = 1 + i
                tv = nt - 2 - i
                # alpha: pa = Eexp^T a ; a = pa * expem_ta
                pa = cp.tile([K, BL], F32, tag="pa")
                nc.tensor.matmul(pa[:], eexp[:], a[:], start=True, stop=True)
                # v: pv = E uv ; uv = pv * expem_tv
                pv = cp.tile([K, BL], F32, tag="pv")
                nc.tensor.matmul(pv[:], eexpT[:], uv[:], start=True, stop=True)
                an = stp.tile([K, BL], F32, tag="alpha")
                nc.vector.tensor_tensor(an[:], pa[:],
                                        expem[:, ta * BL:(ta + 1) * BL],
                                        OP.mult)
                a = an
                un = stp.tile([K, BL], F32, tag="uv")
                nc.vector.tensor_tensor(un[:], pv[:],
                                        expem[:, tv * BL:(tv + 1) * BL],
                                        OP.mult)
                uv = un
                if pendings[0] is not None and i >= pendings[0][1]:
                    a = apply_pending(0, a, "alpha")
                if pendings[1] is not None and i >= pendings[1][1]:
                    uv = apply_pending(1, uv, "uv")
                if i % RESCALE == RESCALE - 1 and i + 3 < nsteps:
                    rescale(0, a, i)
                    rescale(1, uv, i)
            a = apply_pending(0, a, "alpha")
            uv = apply_pending(1, uv, "uv")
            # final v_{TM} = E uv_{TM+1} ; Z = <a, v_TM>
            pv = cp.tile([K, BL], F32, tag="pv")
            nc.tensor.matmul(pv[:], eexpT[:], uv[:], start=True, stop=True)
            q = work.tile([K, BL], F32, tag="q")
            nc.vector.tensor_tensor(q[:], a[:], pv[:], OP.mult)
            psz = sp.tile([1, BL], F32, tag="psz")
            nc.tensor.matmul(psz[:], ones20[:], q[:], start=True, stop=True)
            lnz = work.tile([1, BL], F32, tag="lnz")
            nc.scalar.activation(lnz[:], psz[:], AF.Ln)
            logz = work.tile([1, BL], F32, tag="logz")
            nc.vector.tensor_tensor(logz[:], lnz[:], laccs[0][:], OP.add)
            logz2 = work.tile([1, BL], F32, tag="logz2")
            nc.vector.tensor_tensor(logz2[:], logz[:], laccs[1][:], OP.add)
            nc.sync.dma_start(out=outm[1:2, :], in_=logz2[:])

            # ---- numerator finish ----
            # reduce nacc [K, NCH*BL] (chunk-major) -> [K, BL]
            cur = nacc
            width = NCH * BL
            while width > BL:
                half = width // 2
                red = work.tile([K, half], F32, tag="red")
                nc.vector.tensor_tensor(red[:], cur[:, 0:half],
                                        cur[:, half:width], OP.add)
                cur = red
                width = half
            qe = work.tile([K, BL], F32, tag="qe")
            nc.vector.tensor_tensor(
                qe[:], OHt[:, (nt - 1) * BL:nt * BL],
                end_sb[:, 0:1].to_broadcast([K, BL]), OP.mult)
            stot = work.tile([K, BL], F32, tag="stot")
            nc.vector.tensor_tensor(stot[:], cur[:, 0:BL], qe[:], OP.add)
            pss = sp.tile([1, BL], F32, tag="pss")
            nc.tensor.matmul(pss[:], ones20[:], stot[:], start=True, stop=True)
            score = work.tile([1, BL], F32, tag="score")
            nc.vector.tensor_copy(score[:], pss[:])
            nc.sync.dma_start(out=outm[0:1, :], in_=score[:])

    nc.compile()
    return nc


# ---------------------------------------------------------------------------
# Host side
# ---------------------------------------------------------------------------
_CACHE = {}


def _get_nc(nt):
    if nt not in _CACHE:
        _CACHE[nt] = build(nt)
    return _CACHE[nt]


def prep_inputs(sentences, tags, embed, Wih0, Whh0, b0, Wih1, Whh1, b1,
                Wout, bout, trans, start, end, nt=T):
    """Host-side marshalling: gate-split transposed bf16 weights."""
    f32 = np.float32
    bf16 = ml_dtypes.bfloat16
    gs = np.array([1.0, 1.0, 2.0, 1.0], f32)  # g-gate rows x2 (tanh trick)

    def wT(w, extra):
        # w [4H, D] -> [4, D, H], rows scaled by gs*extra, transposed
        out = np.empty((4, w.shape[1], H), f32)
        for g in range(4):
            out[g] = (w[g * H:(g + 1) * H] * (gs[g] * extra)).T
        return out

    wih0 = np.stack([wT(Wih0[d], 1.0) for d in range(2)]).astype(bf16)
    whh0 = np.stack([wT(Whh0[d], 2.0) for d in range(2)]).astype(bf16)
    b0v = np.stack([(b0[d].reshape(4, H) * gs[:, None]).reshape(4, 1, H)
                    for d in range(2)]).astype(bf16)
    wih1f = np.stack([wT(Wih1[d], 2.0) for d in range(2)])  # [2,4,2H,H]
    wih1 = np.ascontiguousarray(
        wih1f.reshape(2, 4, 2, H, H).transpose(0, 2, 1, 3, 4)).astype(bf16)
    whh1 = np.stack([wT(Whh1[d], 2.0) for d in range(2)]).astype(bf16)
    b1v = np.stack([(b1[d].reshape(4, H) * gs[:, None]).reshape(4, 1, H)
                    for d in range(2)]).astype(bf16)
    woutT = np.stack([np.ascontiguousarray((2.0 * Wout[:, :H]).T),
                      np.ascontiguousarray((2.0 * Wout[:, H:]).T)]).astype(bf16)
    shared = dict(
        embedb=np.ascontiguousarray(embed.astype(bf16)),
        wih0m=wih0, whh0m=whh0, b0m=b0v, wih1m=wih1, whh1m=whh1, b1m=b1v,
        woutm=woutT, boutv=bout.reshape(K, 1).astype(f32),
        transm=trans.astype(f32),
        transTm=np.ascontiguousarray(trans.T).astype(f32),
        transbm=trans.astype(bf16),
        startv=start.reshape(K, 1).astype(f32),
        endv=end.reshape(K, 1).astype(f32),
    )
    in_maps = []
    for c in range(NCORES):
        bsl = slice(c * BL, (c + 1) * BL)
        m = dict(shared)
        m["toks16"] = np.ascontiguousarray(
            sentences[bsl, :nt].astype(np.int16))
        m["tagsf"] = np.ascontiguousarray(
            tags[bsl, :nt].T.astype(f32).reshape(1, BL * nt))  # t-major
        in_maps.append(m)
    return in_maps


def run(inputs_np, nt=T, trace=False):
    nc = _get_nc(nt)
    in_maps = prep_inputs(
        inputs_np["sentences"], inputs_np["tags"], inputs_np["embed"],
        inputs_np["Wih0"], inputs_np["Whh0"], inputs_np["b0"],
        inputs_np["Wih1"], inputs_np["Whh1"], inputs_np["b1"],
        inputs_np["Wout"], inputs_np["bout"], inputs_np["trans"],
        inputs_np["start"], inputs_np["end"], nt=nt)
    res = run_bass_kernel_spmd(nc, in_maps, core_ids=list(range(NCORES)),
                               trace=trace)
    score = np.concatenate([res.results[c]["outm"][0] for c in range(NCORES)])
    logz = np.concatenate([res.results[c]["outm"][1] for c in range(NCORES)])
    loss = -np.mean(score - logz)
    return np.float32(loss), res


def kernel(**inputs):
    inputs_np = {k: np.asarray(v) for k, v in inputs.items()}
    loss, _ = run(inputs_np, nt=T)
    return np.asarray(loss, dtype=np.float32)
